# revision 27
# baseline (speedup 1.0000x reference)
"""Distributed GQA attention + LoRA kernel for one TRN2 chip (8 NeuronCores).

Sharding (tensor-parallel over heads):
  core i owns Q heads 4i..4i+3 and KV head i. wq/wk/wv (+ LoRA B) are sharded
  column-wise over heads; attention is head-local. The output projection is
  sharded over the OUTPUT feature dim d (wo rows): each core computes
  out[:, 512i:512(i+1)] from the full attention output, obtained with one
  AllGather. LoRA-o's rank-16 bottleneck contracts over all heads, so each
  core ships its rank-space partial in the same AllGather and the partials
  are summed on-chip with a selection-matrix matmul.

All activations are kept transposed ([feature, seq]) so every contraction
sits on the partition axis. Weights and x are pre-transposed/cast to bf16 on
the host. RoPE uses a host-side even/odd row permutation of wq/wk so the
rotation becomes partition-half arithmetic. Attention computes S^T = K @ Q^T
directly (no P transposes); softmax is max-free exp (logits are O(10)),
row sums come from a ones-vector matmul over P^T, and 1/sum is applied while
copying the PV result out of PSUM.
"""
import math
import sys
import types

import numpy as np
import ml_dtypes

BF16 = ml_dtypes.bfloat16

DIM = 4096
SEQ = 2048
N_HEADS = 32
N_KV = 8
HD = 128
RANK = 16
NC_ = 8
QH = N_HEADS // NC_  # 4 q heads per core
QO = QH * HD  # 512 local q rows
SCALE = 1.0 / math.sqrt(HD)
AGROWS = QO + RANK  # 528 rows per rank in the all-gather payload

_CACHE = {}
DEBUG_TAPS = False


def _install_hooks():
    if "antenv.axon_hooks" in sys.modules:
        return
    mod = types.ModuleType("antenv.axon_hooks")
    mod._hook = None
    mod.set_axon_ntff_profile_hook = lambda h: setattr(mod, "_hook", h)
    mod.get_axon_ntff_profile_hook = lambda: mod._hook
    sys.modules["antenv.axon_hooks"] = mod
    try:
        from trn_agent_boot.trn_boot import _ntff_profile_via_ctypes

        mod.set_axon_ntff_profile_hook(
            _ntff_profile_via_ctypes("/opt/axon/libaxon_pjrt.so")
        )
    except Exception:
        pass
    from concourse import bass_utils

    bass_utils.upload_artifacts = lambda tmpdir: f"local://{tmpdir}"


def _split_excess_waits(nc, max_waits=1):
    """This walrus build rejects >1 sync wait on CTRL-encoded instructions.
    Move excess waits onto preceding EventSemaphore insts on the same queue."""
    import bass_rust

    n_split = 0
    for f in nc.m.functions:
        for bb in f.blocks:
            new_insts = []
            changed = False
            for ins in bb.instructions:
                si = ins.sync_info
                if si is not None and si.on_wait and len(si.on_wait) > max_waits:
                    waits = list(si.on_wait)
                    excess, keep = waits[:-max_waits], waits[-max_waits:]
                    for i in range(0, len(excess), max_waits):
                        ev = bass_rust.InstEventSemaphore(name=f"WSPLIT-{n_split}")
                        n_split += 1
                        ev.engine = ins.engine
                        ev.sync_info = bass_rust.SyncInfo(
                            on_wait=excess[i : i + max_waits], on_update=[]
                        )
                        new_insts.append(ev)
                    si.on_wait = keep
                    changed = True
                new_insts.append(ins)
            if changed:
                bb.instructions = new_insts
    return n_split


def _build():
    import concourse.bass as bass
    import concourse.mybir as mybir
    import concourse.tile as tile

    f32 = mybir.dt.float32
    bf16 = mybir.dt.bfloat16
    Alu = mybir.AluOpType
    Act = mybir.ActivationFunctionType

    nc = bass.Bass()

    def din(name, shape, dt=bf16):
        return nc.declare_dram_parameter(name, list(shape), dt, isOutput=False)

    xt = din("xt", [DIM, SEQ])
    wq_t = din("wq_t", [DIM, QO])
    wk_t = din("wk_t", [DIM, HD])
    wv_t = din("wv_t", [DIM, HD])
    aqkv_t = din("aqkv_t", [DIM, 96])
    bq_t = din("bq_t", [RANK, QO])
    bk_t = din("bk_t", [RANK, HD])
    bv_t = din("bv_t", [RANK, HD])
    wo_t = din("wo_t", [DIM, QO])
    ao_loc = din("ao_loc", [128, QH * RANK])
    bo_t = din("bo_t", [RANK, QO])
    cos_d = din("cos_d", [128, SEQ], f32)
    sin_d = din("sin_d", [128, SEQ], f32)
    maskt = din("maskt", [128, 4 * 512])
    ident = din("ident", [128, 128])
    ones_c = din("ones_c", [128, 128])
    sum_sel = din("sum_sel", [128, RANK])

    out_d = nc.declare_dram_parameter("out", [QO, SEQ], f32, isOutput=True)

    NSC = SEQ // 512  # 4 seq chunks
    NDT = DIM // 128  # 32 contraction tiles
    NKT = SEQ // 128  # 16 k tiles

    agin = [nc.dram_tensor(f"agin{c}", [AGROWS, 512], bf16) for c in range(NSC)]
    agout = [
        nc.dram_tensor(f"agout{c}", [NC_ * AGROWS, 512], bf16, addr_space="Shared")
        for c in range(NSC)
    ]

    with tile.TileContext(nc) as tc:
        with (
            tc.tile_pool(name="wpool", bufs=1) as wp,
            tc.tile_pool(name="act", bufs=1) as ap_,
            tc.tile_pool(name="stream", bufs=3) as sp,
            tc.tile_pool(name="ps", bufs=1, space="PSUM") as ps,
        ):
            # resident weights, one tile per 128-row contraction block so the
            # first matmul only waits on its own 128KB DMA. wq and wo share
            # SBUF slots (tag bigw{dt}): wo's load waits for phase A's last
            # wq read of that block.
            NG = NDT // 4  # 8 groups of 4 contraction tiles

            def load_grouped(dram, width, tagp):
                tiles = []
                for g in range(NG):
                    t = wp.tile(
                        [128, 4 * width], bf16, tag=f"{tagp}{g}", name=f"{tagp}{g}"
                    )
                    nc.sync.dma_start(
                        t.rearrange("p (t m) -> p t m", t=4),
                        dram[g * 512 : (g + 1) * 512, :].rearrange(
                            "(t p) m -> p t m", p=128
                        ),
                    )
                    tiles.append(t)
                return tiles

            wq_g = load_grouped(wq_t, QO, "bigw")
            wk_g = load_grouped(wk_t, HD, "wk")
            wv_g = load_grouped(wv_t, HD, "wv")
            aq_g = load_grouped(aqkv_t, 96, "aq")
            wq_sb = [wq_g[dt // 4][:, (dt % 4) * QO : (dt % 4 + 1) * QO] for dt in range(NDT)]
            wk_sb = [wk_g[dt // 4][:, (dt % 4) * HD : (dt % 4 + 1) * HD] for dt in range(NDT)]
            wv_sb = [wv_g[dt // 4][:, (dt % 4) * HD : (dt % 4 + 1) * HD] for dt in range(NDT)]
            aq_sb = [aq_g[dt // 4][:, (dt % 4) * 96 : (dt % 4 + 1) * 96] for dt in range(NDT)]
            bq_sb = wp.tile([RANK, QO], bf16)
            nc.sync.dma_start(bq_sb[:], bq_t[:])
            bk_sb = wp.tile([RANK, HD], bf16)
            nc.sync.dma_start(bk_sb[:], bk_t[:])
            bv_sb = wp.tile([RANK, HD], bf16)
            nc.sync.dma_start(bv_sb[:], bv_t[:])
            ao_sb = wp.tile([128, QH * RANK], bf16)
            nc.sync.dma_start(ao_sb[:], ao_loc[:])
            cos_sb = wp.tile([128, SEQ], f32)
            nc.sync.dma_start(cos_sb[:], cos_d[:])
            sin_sb = wp.tile([128, SEQ], f32)
            nc.sync.dma_start(sin_sb[:], sin_d[:])
            mask_sb = wp.tile([128, 4 * 512], bf16)
            nc.sync.dma_start(mask_sb[:], maskt[:])
            ones_sb = wp.tile([128, 128], bf16)
            nc.sync.dma_start(ones_sb[:], ones_c[:])
            bo_sb = wp.tile([RANK, QO], bf16)
            nc.sync.dma_start(bo_sb[:], bo_t[:])
            ss_sb = wp.tile([128, RANK], bf16)
            nc.sync.dma_start(ss_sb[:], sum_sel[:])

            # activations
            qT = [ap_.tile([128, SEQ], bf16, tag=f"qT{j}", name=f"qT{j}") for j in range(QH)]
            kT = ap_.tile([128, SEQ], bf16, tag="kT")
            v_nat = ap_.tile([128, NKT * 128], bf16, tag="vnat")
            attnT = [
                [
                    ap_.tile([128, 512], bf16, tag=f"attnT{j}_{c}", name=f"attnT{j}_{c}")
                    for c in range(NSC)
                ]
                for j in range(QH)
            ]

            # ---- phase A: QKV projection + LoRA (one chunk), no rope ----
            def phase_a_mm(sc):
                s0 = sc * 512
                q_ps = [ps.tile([128, 512], f32, tag=f"q{j}", name=f"q_ps{j}") for j in range(QH)]
                k_ps = ps.tile([128, 512], f32, tag="pk")
                v_ps = ps.tile([128, 512], f32, tag="pv")
                r_ps = ps.tile([96, 512], f32, tag="pr")
                xtg = [None] * NG
                for g in range(NG):
                    xtg[g] = sp.tile([128, 4 * 512], bf16, tag="xt", name="xtg", bufs=3)
                    xt_dma = nc.sync.dma_start(
                        xtg[g].rearrange("p (t m) -> p t m", t=4),
                        xt[g * 512 : (g + 1) * 512, s0 : s0 + 512].rearrange(
                            "(t p) m -> p t m", p=128
                        ),
                    )
                # delay the previous chunk's AllGather until this chunk's xt
                # stream is fully issued: its SDMA churn would starve them.
                if sc - 1 in ag_insts:
                    tile.add_dep_helper(
                        ag_insts.pop(sc - 1).ins, xt_dma.ins,
                        reason="ag after next chunk xt",
                    )
                for dt in range(NDT):
                    xtt = xtg[dt // 4][:, (dt % 4) * 512 : (dt % 4 + 1) * 512]
                    st = dt == 0
                    # r first so its PSUM closes early for the stage-2 copies
                    nc.tensor.matmul(
                        r_ps[:], aq_sb[dt], xtt,
                        start=st, stop=(dt == NDT - 1),
                    )
                    nc.tensor.matmul(
                        k_ps[:], wk_sb[dt], xtt, start=st, stop=False,
                    )
                    nc.tensor.matmul(
                        v_ps[:], wv_sb[dt], xtt, start=st, stop=False,
                    )
                    for j in range(QH):
                        nc.tensor.matmul(
                            q_ps[j][:],
                            wq_sb[dt][:, j * 128 : (j + 1) * 128],
                            xtt,
                            start=st,
                            stop=False,
                        )
                rq = sp.tile([RANK, 512], bf16, tag="rq")
                rk = sp.tile([RANK, 512], bf16, tag="rk")
                rv = sp.tile([RANK, 512], bf16, tag="rv")
                nc.vector.tensor_copy(rq[:], r_ps[0:RANK, :])
                nc.vector.tensor_copy(rk[:], r_ps[32 : 32 + RANK, :])
                nc.vector.tensor_copy(rv[:], r_ps[64 : 64 + RANK, :])
                nc.tensor.matmul(k_ps[:], bk_sb[:], rk[:], start=False, stop=True)
                nc.tensor.matmul(v_ps[:], bv_sb[:], rv[:], start=False, stop=True)
                for j in range(QH):
                    nc.tensor.matmul(
                        q_ps[j][:], bq_sb[:, j * 128 : (j + 1) * 128], rq[:],
                        start=False, stop=True,
                    )
                # v: copy out + transpose to natural via XBAR DMA
                vTc = sp.tile([128, 512], bf16, tag="vT", bufs=2)
                nc.vector.tensor_copy(vTc[:], v_ps[:])
                for kk in range(4):
                    kt = sc * 4 + kk
                    nc.scalar.dma_start_transpose(
                        out=v_nat[:, kt * 128 : (kt + 1) * 128],
                        in_=vTc[:, kk * 128 : (kk + 1) * 128],
                    )
                return q_ps, k_ps

            def rope(src_ps, dst, s0):
                qcp = sp.tile([128, 512], f32, tag="qcp", bufs=2)
                nc.vector.tensor_copy(qcp[:], src_ps[:])
                mc = sp.tile([128, 512], f32, tag="mc", bufs=2)
                nc.vector.tensor_tensor(
                    out=mc[:], in0=qcp[:], in1=cos_sb[:, s0 : s0 + 512], op=Alu.mult
                )
                msh = sp.tile([128, 512], f32, tag="msh", bufs=2)
                nc.vector.tensor_tensor(
                    out=msh[0:64, :], in0=qcp[64:128, :],
                    in1=sin_sb[64:128, s0 : s0 + 512], op=Alu.mult,
                )
                nc.vector.tensor_tensor(
                    out=msh[64:128, :], in0=qcp[0:64, :],
                    in1=sin_sb[0:64, s0 : s0 + 512], op=Alu.mult,
                )
                nc.vector.tensor_tensor(
                    out=dst[0:64, s0 : s0 + 512], in0=mc[0:64, :],
                    in1=msh[0:64, :], op=Alu.subtract,
                )
                nc.vector.tensor_tensor(
                    out=dst[64:128, s0 : s0 + 512], in0=msh[64:128, :],
                    in1=mc[64:128, :], op=Alu.add,
                )

            wo_sb = []
            ag_insts = {}

            def phase_b_head(qc, h, pt):
                q0 = qc * 512
                nkt = 4 * qc + 4
                sum_ps = ps.tile([128, 512], f32, tag="q2", name="sum_ps")
                attn_ps = ps.tile(
                    [128, 512], f32, tag=("q3" if h % 2 == 0 else "c8"), name="attn_ps"
                )

                st_i = [0]

                def emit_s(kt):
                    st_tags = ("q0", "q1", "pk", "pv")
                    st_ps = ps.tile(
                        [128, 512], f32, tag=st_tags[st_i[0] % 4], name="st_ps"
                    )
                    st_i[0] += 1
                    nc.tensor.matmul(
                        st_ps[:],
                        kT[:, kt * 128 : (kt + 1) * 128],
                        qT[h][:, q0 : q0 + 512],
                        start=True,
                        stop=True,
                    )
                    jrel = kt - 4 * qc
                    if jrel >= 0:
                        nc.vector.tensor_tensor(
                            out=st_ps[:], in0=st_ps[:],
                            in1=mask_sb[:, jrel * 512 : (jrel + 1) * 512],
                            op=Alu.add,
                        )
                    nc.scalar.activation(
                        pt[:, kt * 512 : (kt + 1) * 512], st_ps[:],
                        Act.Exp, scale=SCALE,
                    )

                korder = list(range(4 * qc, nkt)) + list(range(0, 4 * qc))

                def emit_pv(kt):
                    first = kt == korder[0]
                    last = kt == korder[-1]
                    nc.tensor.matmul(
                        sum_ps[:], ones_sb[:], pt[:, kt * 512 : (kt + 1) * 512],
                        start=first, stop=last,
                    )
                    nc.tensor.matmul(
                        attn_ps[:], v_nat[:, kt * 128 : (kt + 1) * 128],
                        pt[:, kt * 512 : (kt + 1) * 512],
                        start=first, stop=last,
                    )

                # diagonal (masked) tiles first so their longer mask+exp chain
                # is covered by later S work; PV trails S by 2.
                lag = min(3, nkt - 1)
                for i, kt in enumerate(korder):
                    emit_s(kt)
                    if i >= lag:
                        emit_pv(korder[i - lag])
                for i in range(nkt - lag, nkt):
                    emit_pv(korder[i])
                invb = sp.tile([128, 512], f32, tag="invb", bufs=2)
                # reciprocal on the idle Scalar engine (ACT LUT, ~2^-10 rel —
                # ample for a bf16 result); bass's activation() refuses
                # Reciprocal so emit a Copy and patch the func.
                _i = nc.scalar.activation(
                    invb[:], sum_ps[:], Act.Copy
                )
                _i.ins.func = Act.Reciprocal
                nc.vector.tensor_tensor(
                    out=attnT[h][qc][:], in0=attn_ps[:],
                    in1=invb[:], op=Alu.mult,
                )

            def phase_b_tail(qc):
                q0 = qc * 512
                ro_ps = ps.tile([RANK, 512], f32, tag="q0", name="ro_ps")
                for j in range(QH):
                    nc.tensor.matmul(
                        ro_ps[:], ao_sb[:, j * RANK : (j + 1) * RANK],
                        attnT[j][qc][:],
                        start=(j == 0), stop=(j == QH - 1),
                    )
                ro_loc = sp.tile([RANK, 512], bf16, tag="roloc")
                nc.vector.tensor_copy(ro_loc[:], ro_ps[:])
                for j in range(QH):
                    nc.scalar.dma_start(
                        agin[qc][j * 128 : (j + 1) * 128, :],
                        attnT[j][qc][:],
                    )
                nc.scalar.dma_start(agin[qc][QO:AGROWS, :], ro_loc[:])
                ag_insts[qc] = nc.gpsimd.collective_compute(
                    "AllGather",
                    mybir.AluOpType.bypass,
                    replica_groups=[list(range(NC_))],
                    ins=[agin[qc].ap().opt()],
                    outs=[agout[qc].ap().opt()],
                )

            def phase_c(qc):
                q0 = qc * 512
                # alternate PSUM tag sets so consecutive chunks double-buffer
                otags = ("pk", "pv", "pr", "c8") if qc % 2 == 0 else ("q0", "q1", "q2", "q3")
                rsum_t = sp.tile([128, 512], bf16, tag="rsum", bufs=2)
                for rr in range(NC_):
                    nc.scalar.dma_start(
                        rsum_t[rr * RANK : (rr + 1) * RANK, :],
                        agout[qc][rr * AGROWS + QO : (rr + 1) * AGROWS, :],
                    )
                ro_ps2 = ps.tile([RANK, 512], f32, tag=otags[0], name="ro_ps2")
                nc.tensor.matmul(ro_ps2[:], ss_sb[:], rsum_t[:], start=True, stop=True)
                ro2 = sp.tile([RANK, 512], bf16, tag="ro2sb", bufs=2)
                nc.vector.tensor_copy(ro2[:], ro_ps2[:])
                out_ps = [
                    ps.tile([128, 512], f32, tag=t, name=f"out_ps_{t}")
                    for t in otags
                ]
                for r in range(NC_):
                    attg = sp.tile([128, 4 * 512], bf16, tag="att", bufs=4)
                    nc.sync.dma_start(
                        attg.rearrange("p (t m) -> p t m", t=4),
                        agout[qc][r * AGROWS : r * AGROWS + QO, :].rearrange(
                            "(t p) m -> p t m", p=128
                        ),
                    )
                    for j2 in range(4):
                        ot = r * 4 + j2
                        att = attg[:, j2 * 512 : (j2 + 1) * 512]
                        for j in range(4):
                            nc.tensor.matmul(
                                out_ps[j][:],
                                wo_sb[ot][:, j * 128 : (j + 1) * 128],
                                att,
                                start=(ot == 0),
                                stop=False,
                            )
                for j in range(4):
                    nc.tensor.matmul(
                        out_ps[j][:], bo_sb[:, j * 128 : (j + 1) * 128], ro2[:],
                        start=False, stop=True,
                    )
                for j in range(4):
                    ot_sb = sp.tile([128, 512], f32, tag="osb", bufs=1)
                    nc.vector.tensor_copy(ot_sb[:], out_ps[j][:])
                    nc.scalar.dma_start(
                        out_d[j * 128 : (j + 1) * 128, q0 : q0 + 512], ot_sb[:]
                    )

            # schedule: fine-grained A/B interleave; each head's attention is
            # emitted right after its q tile's rope so the PE never waits for
            # the whole rope burst. AGs fire per chunk; C runs contiguously.
            for sc in range(NSC):
                s0 = sc * 512
                q_ps, k_ps = phase_a_mm(sc)
                if sc == NSC - 1:
                    wo_g = load_grouped(wo_t, QO, "bigw")
                    wo_sb.extend(
                        wo_g[dt // 4][:, (dt % 4) * QO : (dt % 4 + 1) * QO]
                        for dt in range(NDT)
                    )
                rope(k_ps, kT, s0)
                pt = ap_.tile([128, NKT * 512], bf16, tag="pt", name="pt", bufs=1)
                for h in range(QH):
                    rope(q_ps[h], qT[h], s0)
                    phase_b_head(sc, h, pt)
                phase_b_tail(sc)
            for qc in range(NSC):
                phase_c(qc)

    _split_excess_waits(nc)
    return nc


def _host_prep(inputs):
    x = np.asarray(inputs["x"], np.float32)[0]  # [SEQ, DIM]
    wq = np.asarray(inputs["wq"], np.float32)
    wk = np.asarray(inputs["wk"], np.float32)
    wv = np.asarray(inputs["wv"], np.float32)
    wo = np.asarray(inputs["wo"], np.float32)
    fc = np.asarray(inputs["freqs_cos"], np.float32)  # [SEQ, 64]
    fs = np.asarray(inputs["freqs_sin"], np.float32)
    aq = np.asarray(inputs["lora_q_A"], np.float32)
    bq = np.asarray(inputs["lora_q_B"], np.float32)
    ak = np.asarray(inputs["lora_k_A"], np.float32)
    bk = np.asarray(inputs["lora_k_B"], np.float32)
    av = np.asarray(inputs["lora_v_A"], np.float32)
    bv = np.asarray(inputs["lora_v_B"], np.float32)
    ao = np.asarray(inputs["lora_o_A"], np.float32)
    bo = np.asarray(inputs["lora_o_B"], np.float32)

    # even/odd permutation inside each head block (RoPE layout)
    def perm(n_heads):
        p = []
        for h in range(n_heads):
            p.extend(h * HD + np.r_[0:HD:2])
            p.extend(h * HD + np.r_[1:HD:2])
        return np.array(p)

    pq, pk = perm(N_HEADS), perm(N_KV)
    wq_p, wk_p = wq[pq], wk[pk]
    bq_p, bk_p = bq[pq], bk[pk]

    xt = np.ascontiguousarray(x.T).astype(BF16)  # [DIM, SEQ]
    cos_t = np.ascontiguousarray(fc.T)  # [64, SEQ]
    sin_t = np.ascontiguousarray(fs.T)
    cos_d = np.concatenate([cos_t, cos_t], 0).astype(np.float32)
    sin_d = np.concatenate([sin_t, sin_t], 0).astype(np.float32)

    # mask tiles for the S^T diagonal chunks: [128 k, 4*512] f32
    kp = np.arange(128)[:, None]
    qp = np.arange(128)[None, :]
    tri = np.where(kp > qp, np.float32(-1e9), np.float32(0.0))  # [128k,128q]
    maskt = np.zeros((128, 4 * 512), np.float32)
    for jrel in range(4):
        blk = np.zeros((128, 512), np.float32)
        for jj in range(4):
            if jj < jrel:
                blk[:, jj * 128 : (jj + 1) * 128] = -1e9
            elif jj == jrel:
                blk[:, jj * 128 : (jj + 1) * 128] = tri
        maskt[:, jrel * 512 : (jrel + 1) * 512] = blk

    ident = np.eye(128, dtype=BF16)
    ones_c = np.ones((128, 128), BF16)
    sum_sel = np.zeros((128, RANK), np.float32)
    for r in range(NC_):
        for p in range(RANK):
            sum_sel[r * RANK + p, p] = 1.0
    sum_sel = sum_sel.astype(BF16)

    aqkv = np.zeros((96, DIM), np.float32)
    aqkv[0:RANK] = aq
    aqkv[32 : 32 + RANK] = ak
    aqkv[64 : 64 + RANK] = av
    aqkv_t = np.ascontiguousarray(aqkv.T).astype(BF16)

    shared = dict(
        xt=xt,
        aqkv_t=aqkv_t,
        cos_d=cos_d,
        sin_d=sin_d,
        maskt=maskt.astype(BF16),
        ident=ident,
        ones_c=ones_c,
        sum_sel=sum_sel,
    )
    in_maps = []
    for i in range(NC_):
        qs = slice(QO * i, QO * (i + 1))
        ks = slice(HD * i, HD * (i + 1))
        m = dict(shared)
        m["wq_t"] = np.ascontiguousarray(wq_p[qs].T).astype(BF16)
        m["wk_t"] = np.ascontiguousarray(wk_p[ks].T).astype(BF16)
        m["wv_t"] = np.ascontiguousarray(wv[ks].T).astype(BF16)
        m["bq_t"] = np.ascontiguousarray(bq_p[qs].T).astype(BF16)
        m["bk_t"] = np.ascontiguousarray(bk_p[ks].T).astype(BF16)
        m["bv_t"] = np.ascontiguousarray(bv[ks].T).astype(BF16)
        m["wo_t"] = np.ascontiguousarray(wo[qs].T).astype(BF16)
        # A_o^T rows for this core's local heads, laid out per o-tile
        aot = np.ascontiguousarray(ao.T[qs])  # [512, 16]
        m["ao_loc"] = np.ascontiguousarray(
            aot.reshape(QH, 128, RANK).transpose(1, 0, 2).reshape(128, QH * RANK)
        ).astype(BF16)
        m["bo_t"] = np.ascontiguousarray(bo[qs].T).astype(BF16)
        in_maps.append(m)
    return in_maps


def kernel(**inputs):
    _install_hooks()
    from concourse.bass_utils import run_bass_kernel_spmd

    if "nc" not in _CACHE:
        _CACHE["nc"] = _build()
    nc = _CACHE["nc"]

    in_maps = _host_prep(inputs)
    try:
        res = run_bass_kernel_spmd(
            nc, in_maps, core_ids=list(range(NC_)), trace=False
        )
    except Exception:
        # transient device-unrecoverable errors have been observed on the
        # first execution after a fresh compile; one retry clears them.
        res = run_bass_kernel_spmd(
            nc, in_maps, core_ids=list(range(NC_)), trace=False
        )
    out = np.empty((1, SEQ, DIM), np.float32)
    for i in range(NC_):
        out[0, :, QO * i : QO * (i + 1)] = res.results[i]["out"].T
    return out


# revision 28
# speedup vs baseline: 1.0097x; 1.0097x over previous
"""Distributed GQA attention + LoRA kernel for one TRN2 chip (8 NeuronCores).

Sharding (tensor-parallel over heads):
  core i owns Q heads 4i..4i+3 and KV head i. wq/wk/wv (+ LoRA B) are sharded
  column-wise over heads; attention is head-local. The output projection is
  sharded over the OUTPUT feature dim d (wo rows): each core computes
  out[:, 512i:512(i+1)] from the full attention output, obtained with one
  AllGather. LoRA-o's rank-16 bottleneck contracts over all heads, so each
  core ships its rank-space partial in the same AllGather and the partials
  are summed on-chip with a selection-matrix matmul.

All activations are kept transposed ([feature, seq]) so every contraction
sits on the partition axis. Weights and x are pre-transposed/cast to bf16 on
the host. RoPE uses a host-side even/odd row permutation of wq/wk so the
rotation becomes partition-half arithmetic. Attention computes S^T = K @ Q^T
directly (no P transposes); softmax is max-free exp (logits are O(10)),
row sums come from a ones-vector matmul over P^T, and 1/sum is applied while
copying the PV result out of PSUM.
"""
import math
import sys
import types

import numpy as np
import ml_dtypes

BF16 = ml_dtypes.bfloat16

DIM = 4096
SEQ = 2048
N_HEADS = 32
N_KV = 8
HD = 128
RANK = 16
NC_ = 8
QH = N_HEADS // NC_  # 4 q heads per core
QO = QH * HD  # 512 local q rows
SCALE = 1.0 / math.sqrt(HD)
AGROWS = QO + RANK  # 528 rows per rank in the all-gather payload

_CACHE = {}
DEBUG_TAPS = False


def _install_hooks():
    if "antenv.axon_hooks" in sys.modules:
        return
    mod = types.ModuleType("antenv.axon_hooks")
    mod._hook = None
    mod.set_axon_ntff_profile_hook = lambda h: setattr(mod, "_hook", h)
    mod.get_axon_ntff_profile_hook = lambda: mod._hook
    sys.modules["antenv.axon_hooks"] = mod
    try:
        from trn_agent_boot.trn_boot import _ntff_profile_via_ctypes

        mod.set_axon_ntff_profile_hook(
            _ntff_profile_via_ctypes("/opt/axon/libaxon_pjrt.so")
        )
    except Exception:
        pass
    from concourse import bass_utils

    bass_utils.upload_artifacts = lambda tmpdir: f"local://{tmpdir}"


def _split_excess_waits(nc, max_waits=1):
    """This walrus build rejects >1 sync wait on CTRL-encoded instructions.
    Move excess waits onto preceding EventSemaphore insts on the same queue."""
    import bass_rust

    n_split = 0
    for f in nc.m.functions:
        for bb in f.blocks:
            new_insts = []
            changed = False
            for ins in bb.instructions:
                si = ins.sync_info
                if si is not None and si.on_wait and len(si.on_wait) > max_waits:
                    waits = list(si.on_wait)
                    excess, keep = waits[:-max_waits], waits[-max_waits:]
                    for i in range(0, len(excess), max_waits):
                        ev = bass_rust.InstEventSemaphore(name=f"WSPLIT-{n_split}")
                        n_split += 1
                        ev.engine = ins.engine
                        ev.sync_info = bass_rust.SyncInfo(
                            on_wait=excess[i : i + max_waits], on_update=[]
                        )
                        new_insts.append(ev)
                    si.on_wait = keep
                    changed = True
                new_insts.append(ins)
            if changed:
                bb.instructions = new_insts
    return n_split


def _build():
    import concourse.bass as bass
    import concourse.mybir as mybir
    import concourse.tile as tile

    f32 = mybir.dt.float32
    bf16 = mybir.dt.bfloat16
    Alu = mybir.AluOpType
    Act = mybir.ActivationFunctionType

    nc = bass.Bass()

    def din(name, shape, dt=bf16):
        return nc.declare_dram_parameter(name, list(shape), dt, isOutput=False)

    xt = din("xt", [DIM, SEQ])
    wq_t = din("wq_t", [DIM, QO])
    wk_t = din("wk_t", [DIM, HD])
    wv_t = din("wv_t", [DIM, HD])
    aqkv_t = din("aqkv_t", [DIM, 96])
    bq_t = din("bq_t", [RANK, QO])
    bk_t = din("bk_t", [RANK, HD])
    bv_t = din("bv_t", [RANK, HD])
    wo_t = din("wo_t", [DIM, QO])
    ao_loc = din("ao_loc", [128, QH * RANK])
    bo_t = din("bo_t", [RANK, QO])
    cos_d = din("cos_d", [128, SEQ], f32)
    sin_d = din("sin_d", [128, SEQ], f32)
    maskt = din("maskt", [128, 4 * 512])
    ident = din("ident", [128, 128])
    ones_c = din("ones_c", [128, 128])
    sum_sel = din("sum_sel", [128, RANK])

    out_d = nc.declare_dram_parameter("out", [QO, SEQ], f32, isOutput=True)

    NSC = SEQ // 512  # 4 seq chunks
    NDT = DIM // 128  # 32 contraction tiles
    NKT = SEQ // 128  # 16 k tiles

    agin = [nc.dram_tensor(f"agin{c}", [AGROWS, 512], bf16) for c in range(NSC)]
    agout = [
        nc.dram_tensor(f"agout{c}", [NC_ * AGROWS, 512], bf16, addr_space="Shared")
        for c in range(NSC)
    ]

    with tile.TileContext(nc) as tc:
        with (
            tc.tile_pool(name="wpool", bufs=1) as wp,
            tc.tile_pool(name="act", bufs=1) as ap_,
            tc.tile_pool(name="stream", bufs=3) as sp,
            tc.tile_pool(name="ps", bufs=1, space="PSUM") as ps,
        ):
            # resident weights, one tile per 128-row contraction block so the
            # first matmul only waits on its own 128KB DMA. wq and wo share
            # SBUF slots (tag bigw{dt}): wo's load waits for phase A's last
            # wq read of that block.
            NG = NDT // 4  # 8 groups of 4 contraction tiles

            def load_grouped(dram, width, tagp):
                tiles = []
                for g in range(NG):
                    t = wp.tile(
                        [128, 4 * width], bf16, tag=f"{tagp}{g}", name=f"{tagp}{g}"
                    )
                    nc.sync.dma_start(
                        t.rearrange("p (t m) -> p t m", t=4),
                        dram[g * 512 : (g + 1) * 512, :].rearrange(
                            "(t p) m -> p t m", p=128
                        ),
                    )
                    tiles.append(t)
                return tiles

            wq_g = load_grouped(wq_t, QO, "bigw")
            wk_g = load_grouped(wk_t, HD, "wk")
            wv_g = load_grouped(wv_t, HD, "wv")
            aq_g = load_grouped(aqkv_t, 96, "aq")
            wq_sb = [wq_g[dt // 4][:, (dt % 4) * QO : (dt % 4 + 1) * QO] for dt in range(NDT)]
            wk_sb = [wk_g[dt // 4][:, (dt % 4) * HD : (dt % 4 + 1) * HD] for dt in range(NDT)]
            wv_sb = [wv_g[dt // 4][:, (dt % 4) * HD : (dt % 4 + 1) * HD] for dt in range(NDT)]
            aq_sb = [aq_g[dt // 4][:, (dt % 4) * 96 : (dt % 4 + 1) * 96] for dt in range(NDT)]
            bq_sb = wp.tile([RANK, QO], bf16)
            nc.sync.dma_start(bq_sb[:], bq_t[:])
            bk_sb = wp.tile([RANK, HD], bf16)
            nc.sync.dma_start(bk_sb[:], bk_t[:])
            bv_sb = wp.tile([RANK, HD], bf16)
            nc.sync.dma_start(bv_sb[:], bv_t[:])
            ao_sb = wp.tile([128, QH * RANK], bf16)
            nc.sync.dma_start(ao_sb[:], ao_loc[:])
            cos_sb = wp.tile([128, SEQ], f32)
            nc.sync.dma_start(cos_sb[:], cos_d[:])
            sin_sb = wp.tile([128, SEQ], f32)
            nc.sync.dma_start(sin_sb[:], sin_d[:])
            mask_sb = wp.tile([128, 4 * 512], bf16)
            nc.sync.dma_start(mask_sb[:], maskt[:])
            ones_sb = wp.tile([128, 128], bf16)
            nc.sync.dma_start(ones_sb[:], ones_c[:])
            bo_sb = wp.tile([RANK, QO], bf16)
            nc.sync.dma_start(bo_sb[:], bo_t[:])
            ss_sb = wp.tile([128, RANK], bf16)
            nc.sync.dma_start(ss_sb[:], sum_sel[:])

            # activations
            qT = [ap_.tile([128, SEQ], bf16, tag=f"qT{j}", name=f"qT{j}") for j in range(QH)]
            kT = ap_.tile([128, SEQ], bf16, tag="kT")
            v_nat = ap_.tile([128, NKT * 128], bf16, tag="vnat")
            attnT = [
                [
                    ap_.tile([128, 512], bf16, tag=f"attnT{j}_{c}", name=f"attnT{j}_{c}")
                    for c in range(NSC)
                ]
                for j in range(QH)
            ]

            # ---- phase A: QKV projection + LoRA (one chunk), no rope ----
            def phase_a_mm(sc):
                s0 = sc * 512
                q_ps = [ps.tile([128, 512], f32, tag=f"q{j}", name=f"q_ps{j}") for j in range(QH)]
                k_ps = ps.tile([128, 512], f32, tag="pk")
                v_ps = ps.tile([128, 512], f32, tag="pv")
                r_ps = ps.tile([96, 512], f32, tag="pr")
                xtg = [None] * NG
                for g in range(NG):
                    xtg[g] = sp.tile([128, 4 * 512], bf16, tag="xt", name="xtg", bufs=3)
                    xt_dma = nc.sync.dma_start(
                        xtg[g].rearrange("p (t m) -> p t m", t=4),
                        xt[g * 512 : (g + 1) * 512, s0 : s0 + 512].rearrange(
                            "(t p) m -> p t m", p=128
                        ),
                    )
                # delay the previous chunk's AllGather until this chunk's xt
                # stream is fully issued: its SDMA churn would starve them.
                if sc - 1 in ag_insts:
                    tile.add_dep_helper(
                        ag_insts.pop(sc - 1).ins, xt_dma.ins,
                        reason="ag after next chunk xt",
                    )
                for dt in range(NDT):
                    xtt = xtg[dt // 4][:, (dt % 4) * 512 : (dt % 4 + 1) * 512]
                    st = dt == 0
                    # r first so its PSUM closes early for the stage-2 copies
                    nc.tensor.matmul(
                        r_ps[:], aq_sb[dt], xtt,
                        start=st, stop=(dt == NDT - 1),
                    )
                    nc.tensor.matmul(
                        k_ps[:], wk_sb[dt], xtt, start=st, stop=False,
                    )
                    nc.tensor.matmul(
                        v_ps[:], wv_sb[dt], xtt, start=st, stop=False,
                    )
                    for j in range(QH):
                        nc.tensor.matmul(
                            q_ps[j][:],
                            wq_sb[dt][:, j * 128 : (j + 1) * 128],
                            xtt,
                            start=st,
                            stop=False,
                        )
                rq = sp.tile([RANK, 512], bf16, tag="rq")
                rk = sp.tile([RANK, 512], bf16, tag="rk")
                rv = sp.tile([RANK, 512], bf16, tag="rv")
                nc.vector.tensor_copy(rq[:], r_ps[0:RANK, :])
                nc.vector.tensor_copy(rk[:], r_ps[32 : 32 + RANK, :])
                nc.vector.tensor_copy(rv[:], r_ps[64 : 64 + RANK, :])
                nc.tensor.matmul(k_ps[:], bk_sb[:], rk[:], start=False, stop=True)
                nc.tensor.matmul(v_ps[:], bv_sb[:], rv[:], start=False, stop=True)
                for j in range(QH):
                    nc.tensor.matmul(
                        q_ps[j][:], bq_sb[:, j * 128 : (j + 1) * 128], rq[:],
                        start=False, stop=True,
                    )
                # v: copy out + transpose to natural via XBAR DMA
                vTc = sp.tile([128, 512], bf16, tag="vT", bufs=2)
                nc.vector.tensor_copy(vTc[:], v_ps[:])
                for kk in range(4):
                    kt = sc * 4 + kk
                    nc.scalar.dma_start_transpose(
                        out=v_nat[:, kt * 128 : (kt + 1) * 128],
                        in_=vTc[:, kk * 128 : (kk + 1) * 128],
                    )
                return q_ps, k_ps

            def rope(src_ps, dst, s0):
                qcp = sp.tile([128, 512], f32, tag="qcp", bufs=2)
                nc.vector.tensor_copy(qcp[:], src_ps[:])
                mc = sp.tile([128, 512], f32, tag="mc", bufs=2)
                nc.vector.tensor_tensor(
                    out=mc[:], in0=qcp[:], in1=cos_sb[:, s0 : s0 + 512], op=Alu.mult
                )
                msh = sp.tile([128, 512], f32, tag="msh", bufs=2)
                nc.vector.tensor_tensor(
                    out=msh[0:64, :], in0=qcp[64:128, :],
                    in1=sin_sb[64:128, s0 : s0 + 512], op=Alu.mult,
                )
                nc.vector.tensor_tensor(
                    out=msh[64:128, :], in0=qcp[0:64, :],
                    in1=sin_sb[0:64, s0 : s0 + 512], op=Alu.mult,
                )
                nc.vector.tensor_tensor(
                    out=dst[0:64, s0 : s0 + 512], in0=mc[0:64, :],
                    in1=msh[0:64, :], op=Alu.subtract,
                )
                nc.vector.tensor_tensor(
                    out=dst[64:128, s0 : s0 + 512], in0=msh[64:128, :],
                    in1=mc[64:128, :], op=Alu.add,
                )

            wo_sb = []
            ag_insts = {}

            def phase_b_head(qc, h, pt):
                q0 = qc * 512
                nkt = 4 * qc + 4
                sum_ps = ps.tile([128, 512], f32, tag="q2", name="sum_ps")
                attn_ps = ps.tile(
                    [128, 512], f32, tag=("q3" if h % 2 == 0 else "c8"), name="attn_ps"
                )

                st_i = [0]

                def emit_s(kt):
                    st_tags = ("q0", "q1", "pk")
                    st_ps = ps.tile(
                        [128, 512], f32, tag=st_tags[st_i[0] % 3], name="st_ps"
                    )
                    st_i[0] += 1
                    nc.tensor.matmul(
                        st_ps[:],
                        kT[:, kt * 128 : (kt + 1) * 128],
                        qT[h][:, q0 : q0 + 512],
                        start=True,
                        stop=True,
                    )
                    jrel = kt - 4 * qc
                    if jrel >= 0:
                        nc.vector.tensor_tensor(
                            out=st_ps[:], in0=st_ps[:],
                            in1=mask_sb[:, jrel * 512 : (jrel + 1) * 512],
                            op=Alu.add,
                        )
                    nc.scalar.activation(
                        pt[:, kt * 512 : (kt + 1) * 512], st_ps[:],
                        Act.Exp, scale=SCALE,
                    )

                korder = list(range(4 * qc, nkt)) + list(range(0, 4 * qc))

                def emit_pv(kt):
                    first = kt == korder[0]
                    last = kt == korder[-1]
                    nc.tensor.matmul(
                        sum_ps[:], ones_sb[:], pt[:, kt * 512 : (kt + 1) * 512],
                        start=first, stop=last,
                    )
                    nc.tensor.matmul(
                        attn_ps[:], v_nat[:, kt * 128 : (kt + 1) * 128],
                        pt[:, kt * 512 : (kt + 1) * 512],
                        start=first, stop=last,
                    )

                # diagonal (masked) tiles first so their longer mask+exp chain
                # is covered by later S work; PV trails S by 2.
                lag = min(3, nkt - 1)
                for i, kt in enumerate(korder):
                    emit_s(kt)
                    if i >= lag:
                        emit_pv(korder[i - lag])
                for i in range(nkt - lag, nkt):
                    emit_pv(korder[i])
                invb = sp.tile([128, 512], f32, tag="invb", bufs=2)
                # reciprocal on the idle Scalar engine (ACT LUT, ~2^-10 rel —
                # ample for a bf16 result); bass's activation() refuses
                # Reciprocal so emit a Copy and patch the func.
                _i = nc.scalar.activation(
                    invb[:], sum_ps[:], Act.Copy
                )
                _i.ins.func = Act.Reciprocal
                nc.vector.tensor_tensor(
                    out=attnT[h][qc][:], in0=attn_ps[:],
                    in1=invb[:], op=Alu.mult,
                )

            def phase_b_tail(qc):
                q0 = qc * 512
                ro_ps = ps.tile([RANK, 512], f32, tag="q0", name="ro_ps")
                for j in range(QH):
                    nc.tensor.matmul(
                        ro_ps[:], ao_sb[:, j * RANK : (j + 1) * RANK],
                        attnT[j][qc][:],
                        start=(j == 0), stop=(j == QH - 1),
                    )
                ro_loc = sp.tile([RANK, 512], bf16, tag="roloc")
                nc.vector.tensor_copy(ro_loc[:], ro_ps[:])
                for j in range(QH):
                    nc.scalar.dma_start(
                        agin[qc][j * 128 : (j + 1) * 128, :],
                        attnT[j][qc][:],
                    )
                nc.scalar.dma_start(agin[qc][QO:AGROWS, :], ro_loc[:])
                ag_insts[qc] = nc.gpsimd.collective_compute(
                    "AllGather",
                    mybir.AluOpType.bypass,
                    replica_groups=[list(range(NC_))],
                    ins=[agin[qc].ap().opt()],
                    outs=[agout[qc].ap().opt()],
                )

            def phase_c(qc):
                q0 = qc * 512
                # alternate PSUM tag sets so consecutive chunks double-buffer
                otags = ("pk", "pv", "pr", "c8") if qc % 2 == 0 else ("q0", "q1", "q2", "q3")
                rsum_t = sp.tile([128, 512], bf16, tag="rsum", bufs=2)
                for rr in range(NC_):
                    nc.scalar.dma_start(
                        rsum_t[rr * RANK : (rr + 1) * RANK, :],
                        agout[qc][rr * AGROWS + QO : (rr + 1) * AGROWS, :],
                    )
                ro_ps2 = ps.tile([RANK, 512], f32, tag=otags[0], name="ro_ps2")
                nc.tensor.matmul(ro_ps2[:], ss_sb[:], rsum_t[:], start=True, stop=True)
                ro2 = sp.tile([RANK, 512], bf16, tag="ro2sb", bufs=2)
                nc.vector.tensor_copy(ro2[:], ro_ps2[:])
                out_ps = [
                    ps.tile([128, 512], f32, tag=t, name=f"out_ps_{t}")
                    for t in otags
                ]
                for r in range(NC_):
                    attg = sp.tile([128, 4 * 512], bf16, tag="att", bufs=4)
                    nc.sync.dma_start(
                        attg.rearrange("p (t m) -> p t m", t=4),
                        agout[qc][r * AGROWS : r * AGROWS + QO, :].rearrange(
                            "(t p) m -> p t m", p=128
                        ),
                    )
                    for j2 in range(4):
                        ot = r * 4 + j2
                        att = attg[:, j2 * 512 : (j2 + 1) * 512]
                        for j in range(4):
                            nc.tensor.matmul(
                                out_ps[j][:],
                                wo_sb[ot][:, j * 128 : (j + 1) * 128],
                                att,
                                start=(ot == 0),
                                stop=False,
                            )
                for j in range(4):
                    nc.tensor.matmul(
                        out_ps[j][:], bo_sb[:, j * 128 : (j + 1) * 128], ro2[:],
                        start=False, stop=True,
                    )
                for j in range(4):
                    ot_sb = sp.tile([128, 512], f32, tag="osb", bufs=1)
                    nc.vector.tensor_copy(ot_sb[:], out_ps[j][:])
                    nc.scalar.dma_start(
                        out_d[j * 128 : (j + 1) * 128, q0 : q0 + 512], ot_sb[:]
                    )

            # schedule: fine-grained A/B interleave; each head's attention is
            # emitted right after its q tile's rope so the PE never waits for
            # the whole rope burst. AGs fire per chunk; C runs contiguously.
            for sc in range(NSC):
                s0 = sc * 512
                q_ps, k_ps = phase_a_mm(sc)
                if sc == NSC - 1:
                    wo_g = load_grouped(wo_t, QO, "bigw")
                    wo_sb.extend(
                        wo_g[dt // 4][:, (dt % 4) * QO : (dt % 4 + 1) * QO]
                        for dt in range(NDT)
                    )
                rope(k_ps, kT, s0)
                pt = ap_.tile([128, NKT * 512], bf16, tag="pt", name="pt", bufs=1)
                for h in range(QH):
                    rope(q_ps[h], qT[h], s0)
                    phase_b_head(sc, h, pt)
                phase_b_tail(sc)
            for qc in range(NSC):
                phase_c(qc)

    _split_excess_waits(nc)
    return nc


def _host_prep(inputs):
    x = np.asarray(inputs["x"], np.float32)[0]  # [SEQ, DIM]
    wq = np.asarray(inputs["wq"], np.float32)
    wk = np.asarray(inputs["wk"], np.float32)
    wv = np.asarray(inputs["wv"], np.float32)
    wo = np.asarray(inputs["wo"], np.float32)
    fc = np.asarray(inputs["freqs_cos"], np.float32)  # [SEQ, 64]
    fs = np.asarray(inputs["freqs_sin"], np.float32)
    aq = np.asarray(inputs["lora_q_A"], np.float32)
    bq = np.asarray(inputs["lora_q_B"], np.float32)
    ak = np.asarray(inputs["lora_k_A"], np.float32)
    bk = np.asarray(inputs["lora_k_B"], np.float32)
    av = np.asarray(inputs["lora_v_A"], np.float32)
    bv = np.asarray(inputs["lora_v_B"], np.float32)
    ao = np.asarray(inputs["lora_o_A"], np.float32)
    bo = np.asarray(inputs["lora_o_B"], np.float32)

    # even/odd permutation inside each head block (RoPE layout)
    def perm(n_heads):
        p = []
        for h in range(n_heads):
            p.extend(h * HD + np.r_[0:HD:2])
            p.extend(h * HD + np.r_[1:HD:2])
        return np.array(p)

    pq, pk = perm(N_HEADS), perm(N_KV)
    wq_p, wk_p = wq[pq], wk[pk]
    bq_p, bk_p = bq[pq], bk[pk]

    xt = np.ascontiguousarray(x.T).astype(BF16)  # [DIM, SEQ]
    cos_t = np.ascontiguousarray(fc.T)  # [64, SEQ]
    sin_t = np.ascontiguousarray(fs.T)
    cos_d = np.concatenate([cos_t, cos_t], 0).astype(np.float32)
    sin_d = np.concatenate([sin_t, sin_t], 0).astype(np.float32)

    # mask tiles for the S^T diagonal chunks: [128 k, 4*512] f32
    kp = np.arange(128)[:, None]
    qp = np.arange(128)[None, :]
    tri = np.where(kp > qp, np.float32(-1e9), np.float32(0.0))  # [128k,128q]
    maskt = np.zeros((128, 4 * 512), np.float32)
    for jrel in range(4):
        blk = np.zeros((128, 512), np.float32)
        for jj in range(4):
            if jj < jrel:
                blk[:, jj * 128 : (jj + 1) * 128] = -1e9
            elif jj == jrel:
                blk[:, jj * 128 : (jj + 1) * 128] = tri
        maskt[:, jrel * 512 : (jrel + 1) * 512] = blk

    ident = np.eye(128, dtype=BF16)
    ones_c = np.ones((128, 128), BF16)
    sum_sel = np.zeros((128, RANK), np.float32)
    for r in range(NC_):
        for p in range(RANK):
            sum_sel[r * RANK + p, p] = 1.0
    sum_sel = sum_sel.astype(BF16)

    aqkv = np.zeros((96, DIM), np.float32)
    aqkv[0:RANK] = aq
    aqkv[32 : 32 + RANK] = ak
    aqkv[64 : 64 + RANK] = av
    aqkv_t = np.ascontiguousarray(aqkv.T).astype(BF16)

    shared = dict(
        xt=xt,
        aqkv_t=aqkv_t,
        cos_d=cos_d,
        sin_d=sin_d,
        maskt=maskt.astype(BF16),
        ident=ident,
        ones_c=ones_c,
        sum_sel=sum_sel,
    )
    in_maps = []
    for i in range(NC_):
        qs = slice(QO * i, QO * (i + 1))
        ks = slice(HD * i, HD * (i + 1))
        m = dict(shared)
        m["wq_t"] = np.ascontiguousarray(wq_p[qs].T).astype(BF16)
        m["wk_t"] = np.ascontiguousarray(wk_p[ks].T).astype(BF16)
        m["wv_t"] = np.ascontiguousarray(wv[ks].T).astype(BF16)
        m["bq_t"] = np.ascontiguousarray(bq_p[qs].T).astype(BF16)
        m["bk_t"] = np.ascontiguousarray(bk_p[ks].T).astype(BF16)
        m["bv_t"] = np.ascontiguousarray(bv[ks].T).astype(BF16)
        m["wo_t"] = np.ascontiguousarray(wo[qs].T).astype(BF16)
        # A_o^T rows for this core's local heads, laid out per o-tile
        aot = np.ascontiguousarray(ao.T[qs])  # [512, 16]
        m["ao_loc"] = np.ascontiguousarray(
            aot.reshape(QH, 128, RANK).transpose(1, 0, 2).reshape(128, QH * RANK)
        ).astype(BF16)
        m["bo_t"] = np.ascontiguousarray(bo[qs].T).astype(BF16)
        in_maps.append(m)
    return in_maps


def kernel(**inputs):
    _install_hooks()
    from concourse.bass_utils import run_bass_kernel_spmd

    if "nc" not in _CACHE:
        _CACHE["nc"] = _build()
    nc = _CACHE["nc"]

    in_maps = _host_prep(inputs)
    try:
        res = run_bass_kernel_spmd(
            nc, in_maps, core_ids=list(range(NC_)), trace=False
        )
    except Exception:
        # transient device-unrecoverable errors have been observed on the
        # first execution after a fresh compile; one retry clears them.
        res = run_bass_kernel_spmd(
            nc, in_maps, core_ids=list(range(NC_)), trace=False
        )
    out = np.empty((1, SEQ, DIM), np.float32)
    for i in range(NC_):
        out[0, :, QO * i : QO * (i + 1)] = res.results[i]["out"].T
    return out


# revision 29
# speedup vs baseline: 1.0262x; 1.0164x over previous
"""Distributed GQA attention + LoRA kernel for one TRN2 chip (8 NeuronCores).

Sharding (tensor-parallel over heads):
  core i owns Q heads 4i..4i+3 and KV head i. wq/wk/wv (+ LoRA B) are sharded
  column-wise over heads; attention is head-local. The output projection is
  sharded over the OUTPUT feature dim d (wo rows): each core computes
  out[:, 512i:512(i+1)] from the full attention output, obtained with one
  AllGather. LoRA-o's rank-16 bottleneck contracts over all heads, so each
  core ships its rank-space partial in the same AllGather and the partials
  are summed on-chip with a selection-matrix matmul.

All activations are kept transposed ([feature, seq]) so every contraction
sits on the partition axis. Weights and x are pre-transposed/cast to bf16 on
the host. RoPE uses a host-side even/odd row permutation of wq/wk so the
rotation becomes partition-half arithmetic. Attention computes S^T = K @ Q^T
directly (no P transposes); softmax is max-free exp (logits are O(10)),
row sums come from a ones-vector matmul over P^T, and 1/sum is applied while
copying the PV result out of PSUM.
"""
import math
import sys
import types

import numpy as np
import ml_dtypes

BF16 = ml_dtypes.bfloat16

DIM = 4096
SEQ = 2048
N_HEADS = 32
N_KV = 8
HD = 128
RANK = 16
NC_ = 8
QH = N_HEADS // NC_  # 4 q heads per core
QO = QH * HD  # 512 local q rows
SCALE = 1.0 / math.sqrt(HD)
AGROWS = QO + RANK  # 528 rows per rank in the all-gather payload

_CACHE = {}
DEBUG_TAPS = False


def _install_hooks():
    if "antenv.axon_hooks" in sys.modules:
        return
    mod = types.ModuleType("antenv.axon_hooks")
    mod._hook = None
    mod.set_axon_ntff_profile_hook = lambda h: setattr(mod, "_hook", h)
    mod.get_axon_ntff_profile_hook = lambda: mod._hook
    sys.modules["antenv.axon_hooks"] = mod
    try:
        from trn_agent_boot.trn_boot import _ntff_profile_via_ctypes

        mod.set_axon_ntff_profile_hook(
            _ntff_profile_via_ctypes("/opt/axon/libaxon_pjrt.so")
        )
    except Exception:
        pass
    from concourse import bass_utils

    bass_utils.upload_artifacts = lambda tmpdir: f"local://{tmpdir}"


def _split_excess_waits(nc, max_waits=1):
    """This walrus build rejects >1 sync wait on CTRL-encoded instructions.
    Move excess waits onto preceding EventSemaphore insts on the same queue."""
    import bass_rust

    n_split = 0
    for f in nc.m.functions:
        for bb in f.blocks:
            new_insts = []
            changed = False
            for ins in bb.instructions:
                si = ins.sync_info
                if si is not None and si.on_wait and len(si.on_wait) > max_waits:
                    waits = list(si.on_wait)
                    excess, keep = waits[:-max_waits], waits[-max_waits:]
                    for i in range(0, len(excess), max_waits):
                        ev = bass_rust.InstEventSemaphore(name=f"WSPLIT-{n_split}")
                        n_split += 1
                        ev.engine = ins.engine
                        ev.sync_info = bass_rust.SyncInfo(
                            on_wait=excess[i : i + max_waits], on_update=[]
                        )
                        new_insts.append(ev)
                    si.on_wait = keep
                    changed = True
                new_insts.append(ins)
            if changed:
                bb.instructions = new_insts
    return n_split


def _build():
    import concourse.bass as bass
    import concourse.mybir as mybir
    import concourse.tile as tile

    f32 = mybir.dt.float32
    bf16 = mybir.dt.bfloat16
    Alu = mybir.AluOpType
    Act = mybir.ActivationFunctionType

    nc = bass.Bass()

    def din(name, shape, dt=bf16):
        return nc.declare_dram_parameter(name, list(shape), dt, isOutput=False)

    xt = din("xt", [DIM, SEQ])
    wq_t = din("wq_t", [DIM, QO])
    wk_t = din("wk_t", [DIM, HD])
    wv_t = din("wv_t", [DIM, HD])
    aqkv_t = din("aqkv_t", [DIM, 96])
    bq_t = din("bq_t", [RANK, QO])
    bk_t = din("bk_t", [RANK, HD])
    bv_t = din("bv_t", [RANK, HD])
    wo_t = din("wo_t", [DIM, QO])
    ao_loc = din("ao_loc", [128, QH * RANK])
    bo_t = din("bo_t", [RANK, QO])
    cos_d = din("cos_d", [128, SEQ], f32)
    sin_d = din("sin_d", [128, SEQ], f32)
    maskt = din("maskt", [128, 4 * 512])
    ident = din("ident", [128, 128])
    ones_c = din("ones_c", [128, 128])
    sum_sel = din("sum_sel", [128, RANK])

    out_d = nc.declare_dram_parameter("out", [QO, SEQ], f32, isOutput=True)

    NSC = SEQ // 512  # 4 seq chunks
    NDT = DIM // 128  # 32 contraction tiles
    NKT = SEQ // 128  # 16 k tiles

    agin = [nc.dram_tensor(f"agin{c}", [AGROWS, 512], bf16) for c in range(NSC)]
    agout = [
        nc.dram_tensor(f"agout{c}", [NC_ * AGROWS, 512], bf16, addr_space="Shared")
        for c in range(NSC)
    ]

    with tile.TileContext(nc) as tc:
        with (
            tc.tile_pool(name="wpool", bufs=1) as wp,
            tc.tile_pool(name="act", bufs=1) as ap_,
            tc.tile_pool(name="stream", bufs=3) as sp,
            tc.tile_pool(name="ps", bufs=1, space="PSUM") as ps,
        ):
            # resident weights, one tile per 128-row contraction block so the
            # first matmul only waits on its own 128KB DMA. wq and wo share
            # SBUF slots (tag bigw{dt}): wo's load waits for phase A's last
            # wq read of that block.
            NG = NDT // 4  # 8 groups of 4 contraction tiles

            def load_grouped(dram, width, tagp):
                tiles = []
                for g in range(NG):
                    t = wp.tile(
                        [128, 4 * width], bf16, tag=f"{tagp}{g}", name=f"{tagp}{g}"
                    )
                    nc.sync.dma_start(
                        t.rearrange("p (t m) -> p t m", t=4),
                        dram[g * 512 : (g + 1) * 512, :].rearrange(
                            "(t p) m -> p t m", p=128
                        ),
                    )
                    tiles.append(t)
                return tiles

            wq_g = load_grouped(wq_t, QO, "bigw")
            wk_g = load_grouped(wk_t, HD, "wk")
            wv_g = load_grouped(wv_t, HD, "wv")
            aq_g = load_grouped(aqkv_t, 96, "aq")
            wq_sb = [wq_g[dt // 4][:, (dt % 4) * QO : (dt % 4 + 1) * QO] for dt in range(NDT)]
            wk_sb = [wk_g[dt // 4][:, (dt % 4) * HD : (dt % 4 + 1) * HD] for dt in range(NDT)]
            wv_sb = [wv_g[dt // 4][:, (dt % 4) * HD : (dt % 4 + 1) * HD] for dt in range(NDT)]
            aq_sb = [aq_g[dt // 4][:, (dt % 4) * 96 : (dt % 4 + 1) * 96] for dt in range(NDT)]
            bq_sb = wp.tile([RANK, QO], bf16)
            nc.sync.dma_start(bq_sb[:], bq_t[:])
            bk_sb = wp.tile([RANK, HD], bf16)
            nc.sync.dma_start(bk_sb[:], bk_t[:])
            bv_sb = wp.tile([RANK, HD], bf16)
            nc.sync.dma_start(bv_sb[:], bv_t[:])
            ao_sb = wp.tile([128, QH * RANK], bf16)
            nc.sync.dma_start(ao_sb[:], ao_loc[:])
            cos_sb = wp.tile([128, SEQ], f32)
            nc.sync.dma_start(cos_sb[:], cos_d[:])
            sin_sb = wp.tile([128, SEQ], f32)
            nc.sync.dma_start(sin_sb[:], sin_d[:])
            mask_sb = wp.tile([128, 4 * 512], bf16)
            nc.sync.dma_start(mask_sb[:], maskt[:])
            ones_sb = wp.tile([128, 128], bf16)
            nc.sync.dma_start(ones_sb[:], ones_c[:])
            bo_sb = wp.tile([RANK, QO], bf16)
            nc.sync.dma_start(bo_sb[:], bo_t[:])
            ss_sb = wp.tile([128, RANK], bf16)
            nc.sync.dma_start(ss_sb[:], sum_sel[:])

            # activations
            qT = [ap_.tile([128, SEQ], bf16, tag=f"qT{j}", name=f"qT{j}") for j in range(QH)]
            kT = ap_.tile([128, SEQ], bf16, tag="kT")
            v_nat = ap_.tile([128, NKT * 128], bf16, tag="vnat")
            attnT = [
                [
                    ap_.tile([128, 512], bf16, tag=f"attnT{j}_{c}", name=f"attnT{j}_{c}")
                    for c in range(NSC)
                ]
                for j in range(QH)
            ]

            # ---- phase A: QKV projection + LoRA (one chunk), no rope ----
            def phase_a_mm(sc):
                s0 = sc * 512
                q_ps = [ps.tile([128, 512], f32, tag=f"q{j}", name=f"q_ps{j}") for j in range(QH)]
                k_ps = ps.tile([128, 512], f32, tag="pk")
                v_ps = ps.tile([128, 512], f32, tag="pv")
                r_ps = ps.tile([96, 512], f32, tag="pr")
                xtg = [None] * NG
                for g in range(NG):
                    xtg[g] = sp.tile([128, 4 * 512], bf16, tag="xt", name="xtg", bufs=3)
                    xt_dma = nc.sync.dma_start(
                        xtg[g].rearrange("p (t m) -> p t m", t=4),
                        xt[g * 512 : (g + 1) * 512, s0 : s0 + 512].rearrange(
                            "(t p) m -> p t m", p=128
                        ),
                    )

                for dt in range(NDT):
                    xtt = xtg[dt // 4][:, (dt % 4) * 512 : (dt % 4 + 1) * 512]
                    st = dt == 0
                    # r first so its PSUM closes early for the stage-2 copies
                    nc.tensor.matmul(
                        r_ps[:], aq_sb[dt], xtt,
                        start=st, stop=(dt == NDT - 1),
                    )
                    nc.tensor.matmul(
                        k_ps[:], wk_sb[dt], xtt, start=st, stop=False,
                    )
                    nc.tensor.matmul(
                        v_ps[:], wv_sb[dt], xtt, start=st, stop=False,
                    )
                    for j in range(QH):
                        nc.tensor.matmul(
                            q_ps[j][:],
                            wq_sb[dt][:, j * 128 : (j + 1) * 128],
                            xtt,
                            start=st,
                            stop=False,
                        )
                rq = sp.tile([RANK, 512], bf16, tag="rq")
                rk = sp.tile([RANK, 512], bf16, tag="rk")
                rv = sp.tile([RANK, 512], bf16, tag="rv")
                nc.vector.tensor_copy(rq[:], r_ps[0:RANK, :])
                nc.vector.tensor_copy(rk[:], r_ps[32 : 32 + RANK, :])
                nc.vector.tensor_copy(rv[:], r_ps[64 : 64 + RANK, :])
                nc.tensor.matmul(k_ps[:], bk_sb[:], rk[:], start=False, stop=True)
                nc.tensor.matmul(v_ps[:], bv_sb[:], rv[:], start=False, stop=True)
                for j in range(QH):
                    nc.tensor.matmul(
                        q_ps[j][:], bq_sb[:, j * 128 : (j + 1) * 128], rq[:],
                        start=False, stop=True,
                    )
                # v: copy out + transpose to natural via XBAR DMA
                vTc = sp.tile([128, 512], bf16, tag="vT", bufs=2)
                nc.vector.tensor_copy(vTc[:], v_ps[:])
                for kk in range(4):
                    kt = sc * 4 + kk
                    nc.scalar.dma_start_transpose(
                        out=v_nat[:, kt * 128 : (kt + 1) * 128],
                        in_=vTc[:, kk * 128 : (kk + 1) * 128],
                    )
                return q_ps, k_ps

            def rope(src_ps, dst, s0):
                qcp = sp.tile([128, 512], f32, tag="qcp", bufs=2)
                nc.vector.tensor_copy(qcp[:], src_ps[:])
                mc = sp.tile([128, 512], f32, tag="mc", bufs=2)
                nc.vector.tensor_tensor(
                    out=mc[:], in0=qcp[:], in1=cos_sb[:, s0 : s0 + 512], op=Alu.mult
                )
                msh = sp.tile([128, 512], f32, tag="msh", bufs=2)
                nc.vector.tensor_tensor(
                    out=msh[0:64, :], in0=qcp[64:128, :],
                    in1=sin_sb[64:128, s0 : s0 + 512], op=Alu.mult,
                )
                nc.vector.tensor_tensor(
                    out=msh[64:128, :], in0=qcp[0:64, :],
                    in1=sin_sb[0:64, s0 : s0 + 512], op=Alu.mult,
                )
                nc.vector.tensor_tensor(
                    out=dst[0:64, s0 : s0 + 512], in0=mc[0:64, :],
                    in1=msh[0:64, :], op=Alu.subtract,
                )
                nc.vector.tensor_tensor(
                    out=dst[64:128, s0 : s0 + 512], in0=msh[64:128, :],
                    in1=mc[64:128, :], op=Alu.add,
                )

            wo_sb = []
            ag_insts = {}

            def phase_b_head(qc, h, pt):
                q0 = qc * 512
                nkt = 4 * qc + 4
                sum_ps = ps.tile([128, 512], f32, tag="q2", name="sum_ps")
                attn_ps = ps.tile(
                    [128, 512], f32, tag=("q3" if h % 2 == 0 else "c8"), name="attn_ps"
                )

                st_i = [0]

                def emit_s(kt):
                    st_tags = ("q0", "q1", "pk")
                    st_ps = ps.tile(
                        [128, 512], f32, tag=st_tags[st_i[0] % 3], name="st_ps"
                    )
                    st_i[0] += 1
                    nc.tensor.matmul(
                        st_ps[:],
                        kT[:, kt * 128 : (kt + 1) * 128],
                        qT[h][:, q0 : q0 + 512],
                        start=True,
                        stop=True,
                    )
                    jrel = kt - 4 * qc
                    if jrel >= 0:
                        nc.vector.tensor_tensor(
                            out=st_ps[:], in0=st_ps[:],
                            in1=mask_sb[:, jrel * 512 : (jrel + 1) * 512],
                            op=Alu.add,
                        )
                    nc.scalar.activation(
                        pt[:, kt * 512 : (kt + 1) * 512], st_ps[:],
                        Act.Exp, scale=SCALE,
                    )

                korder = list(range(4 * qc, nkt)) + list(range(0, 4 * qc))

                def emit_pv(kt):
                    first = kt == korder[0]
                    last = kt == korder[-1]
                    nc.tensor.matmul(
                        sum_ps[:], ones_sb[:], pt[:, kt * 512 : (kt + 1) * 512],
                        start=first, stop=last,
                    )
                    nc.tensor.matmul(
                        attn_ps[:], v_nat[:, kt * 128 : (kt + 1) * 128],
                        pt[:, kt * 512 : (kt + 1) * 512],
                        start=first, stop=last,
                    )

                # diagonal (masked) tiles first so their longer mask+exp chain
                # is covered by later S work; PV trails S by 2.
                lag = min(3, nkt - 1)
                for i, kt in enumerate(korder):
                    emit_s(kt)
                    if i >= lag:
                        emit_pv(korder[i - lag])
                for i in range(nkt - lag, nkt):
                    emit_pv(korder[i])
                invb = sp.tile([128, 512], f32, tag="invb", bufs=2)
                # reciprocal on the idle Scalar engine (ACT LUT, ~2^-10 rel —
                # ample for a bf16 result); bass's activation() refuses
                # Reciprocal so emit a Copy and patch the func.
                _i = nc.scalar.activation(
                    invb[:], sum_ps[:], Act.Copy
                )
                _i.ins.func = Act.Reciprocal
                nc.vector.tensor_tensor(
                    out=attnT[h][qc][:], in0=attn_ps[:],
                    in1=invb[:], op=Alu.mult,
                )

            def phase_b_tail(qc):
                q0 = qc * 512
                ro_ps = ps.tile([RANK, 512], f32, tag="q0", name="ro_ps")
                for j in range(QH):
                    nc.tensor.matmul(
                        ro_ps[:], ao_sb[:, j * RANK : (j + 1) * RANK],
                        attnT[j][qc][:],
                        start=(j == 0), stop=(j == QH - 1),
                    )
                ro_loc = sp.tile([RANK, 512], bf16, tag="roloc")
                nc.vector.tensor_copy(ro_loc[:], ro_ps[:])
                for j in range(QH):
                    nc.scalar.dma_start(
                        agin[qc][j * 128 : (j + 1) * 128, :],
                        attnT[j][qc][:],
                    )
                nc.scalar.dma_start(agin[qc][QO:AGROWS, :], ro_loc[:])
                ag_insts[qc] = nc.gpsimd.collective_compute(
                    "AllGather",
                    mybir.AluOpType.bypass,
                    replica_groups=[list(range(NC_))],
                    ins=[agin[qc].ap().opt()],
                    outs=[agout[qc].ap().opt()],
                )

            def phase_c(qc):
                q0 = qc * 512
                # alternate PSUM tag sets so consecutive chunks double-buffer
                otags = ("pk", "pv", "pr", "c8") if qc % 2 == 0 else ("q0", "q1", "q2", "q3")
                rsum_t = sp.tile([128, 512], bf16, tag="rsum", bufs=2)
                for rr in range(NC_):
                    nc.scalar.dma_start(
                        rsum_t[rr * RANK : (rr + 1) * RANK, :],
                        agout[qc][rr * AGROWS + QO : (rr + 1) * AGROWS, :],
                    )
                ro_ps2 = ps.tile([RANK, 512], f32, tag=otags[0], name="ro_ps2")
                nc.tensor.matmul(ro_ps2[:], ss_sb[:], rsum_t[:], start=True, stop=True)
                ro2 = sp.tile([RANK, 512], bf16, tag="ro2sb", bufs=2)
                nc.vector.tensor_copy(ro2[:], ro_ps2[:])
                out_ps = [
                    ps.tile([128, 512], f32, tag=t, name=f"out_ps_{t}")
                    for t in otags
                ]
                for r in range(NC_):
                    attg = sp.tile([128, 4 * 512], bf16, tag="att", bufs=4)
                    nc.sync.dma_start(
                        attg.rearrange("p (t m) -> p t m", t=4),
                        agout[qc][r * AGROWS : r * AGROWS + QO, :].rearrange(
                            "(t p) m -> p t m", p=128
                        ),
                    )
                    for j2 in range(4):
                        ot = r * 4 + j2
                        att = attg[:, j2 * 512 : (j2 + 1) * 512]
                        for j in range(4):
                            nc.tensor.matmul(
                                out_ps[j][:],
                                wo_sb[ot][:, j * 128 : (j + 1) * 128],
                                att,
                                start=(ot == 0),
                                stop=False,
                            )
                for j in range(4):
                    nc.tensor.matmul(
                        out_ps[j][:], bo_sb[:, j * 128 : (j + 1) * 128], ro2[:],
                        start=False, stop=True,
                    )
                for j in range(4):
                    ot_sb = sp.tile([128, 512], f32, tag="osb", bufs=1)
                    nc.vector.tensor_copy(ot_sb[:], out_ps[j][:])
                    nc.scalar.dma_start(
                        out_d[j * 128 : (j + 1) * 128, q0 : q0 + 512], ot_sb[:]
                    )

            # schedule: fine-grained A/B interleave; each head's attention is
            # emitted right after its q tile's rope so the PE never waits for
            # the whole rope burst. AGs fire per chunk; C runs contiguously.
            for sc in range(NSC):
                s0 = sc * 512
                q_ps, k_ps = phase_a_mm(sc)
                if sc == NSC - 1:
                    wo_g = load_grouped(wo_t, QO, "bigw")
                    wo_sb.extend(
                        wo_g[dt // 4][:, (dt % 4) * QO : (dt % 4 + 1) * QO]
                        for dt in range(NDT)
                    )
                rope(k_ps, kT, s0)
                pt = ap_.tile([128, NKT * 512], bf16, tag="pt", name="pt", bufs=1)
                for h in range(QH):
                    rope(q_ps[h], qT[h], s0)
                    phase_b_head(sc, h, pt)
                phase_b_tail(sc)
            for qc in range(NSC):
                phase_c(qc)

    _split_excess_waits(nc)
    return nc


def _host_prep(inputs):
    x = np.asarray(inputs["x"], np.float32)[0]  # [SEQ, DIM]
    wq = np.asarray(inputs["wq"], np.float32)
    wk = np.asarray(inputs["wk"], np.float32)
    wv = np.asarray(inputs["wv"], np.float32)
    wo = np.asarray(inputs["wo"], np.float32)
    fc = np.asarray(inputs["freqs_cos"], np.float32)  # [SEQ, 64]
    fs = np.asarray(inputs["freqs_sin"], np.float32)
    aq = np.asarray(inputs["lora_q_A"], np.float32)
    bq = np.asarray(inputs["lora_q_B"], np.float32)
    ak = np.asarray(inputs["lora_k_A"], np.float32)
    bk = np.asarray(inputs["lora_k_B"], np.float32)
    av = np.asarray(inputs["lora_v_A"], np.float32)
    bv = np.asarray(inputs["lora_v_B"], np.float32)
    ao = np.asarray(inputs["lora_o_A"], np.float32)
    bo = np.asarray(inputs["lora_o_B"], np.float32)

    # even/odd permutation inside each head block (RoPE layout)
    def perm(n_heads):
        p = []
        for h in range(n_heads):
            p.extend(h * HD + np.r_[0:HD:2])
            p.extend(h * HD + np.r_[1:HD:2])
        return np.array(p)

    pq, pk = perm(N_HEADS), perm(N_KV)
    wq_p, wk_p = wq[pq], wk[pk]
    bq_p, bk_p = bq[pq], bk[pk]

    xt = np.ascontiguousarray(x.T).astype(BF16)  # [DIM, SEQ]
    cos_t = np.ascontiguousarray(fc.T)  # [64, SEQ]
    sin_t = np.ascontiguousarray(fs.T)
    cos_d = np.concatenate([cos_t, cos_t], 0).astype(np.float32)
    sin_d = np.concatenate([sin_t, sin_t], 0).astype(np.float32)

    # mask tiles for the S^T diagonal chunks: [128 k, 4*512] f32
    kp = np.arange(128)[:, None]
    qp = np.arange(128)[None, :]
    tri = np.where(kp > qp, np.float32(-1e9), np.float32(0.0))  # [128k,128q]
    maskt = np.zeros((128, 4 * 512), np.float32)
    for jrel in range(4):
        blk = np.zeros((128, 512), np.float32)
        for jj in range(4):
            if jj < jrel:
                blk[:, jj * 128 : (jj + 1) * 128] = -1e9
            elif jj == jrel:
                blk[:, jj * 128 : (jj + 1) * 128] = tri
        maskt[:, jrel * 512 : (jrel + 1) * 512] = blk

    ident = np.eye(128, dtype=BF16)
    ones_c = np.ones((128, 128), BF16)
    sum_sel = np.zeros((128, RANK), np.float32)
    for r in range(NC_):
        for p in range(RANK):
            sum_sel[r * RANK + p, p] = 1.0
    sum_sel = sum_sel.astype(BF16)

    aqkv = np.zeros((96, DIM), np.float32)
    aqkv[0:RANK] = aq
    aqkv[32 : 32 + RANK] = ak
    aqkv[64 : 64 + RANK] = av
    aqkv_t = np.ascontiguousarray(aqkv.T).astype(BF16)

    shared = dict(
        xt=xt,
        aqkv_t=aqkv_t,
        cos_d=cos_d,
        sin_d=sin_d,
        maskt=maskt.astype(BF16),
        ident=ident,
        ones_c=ones_c,
        sum_sel=sum_sel,
    )
    in_maps = []
    for i in range(NC_):
        qs = slice(QO * i, QO * (i + 1))
        ks = slice(HD * i, HD * (i + 1))
        m = dict(shared)
        m["wq_t"] = np.ascontiguousarray(wq_p[qs].T).astype(BF16)
        m["wk_t"] = np.ascontiguousarray(wk_p[ks].T).astype(BF16)
        m["wv_t"] = np.ascontiguousarray(wv[ks].T).astype(BF16)
        m["bq_t"] = np.ascontiguousarray(bq_p[qs].T).astype(BF16)
        m["bk_t"] = np.ascontiguousarray(bk_p[ks].T).astype(BF16)
        m["bv_t"] = np.ascontiguousarray(bv[ks].T).astype(BF16)
        m["wo_t"] = np.ascontiguousarray(wo[qs].T).astype(BF16)
        # A_o^T rows for this core's local heads, laid out per o-tile
        aot = np.ascontiguousarray(ao.T[qs])  # [512, 16]
        m["ao_loc"] = np.ascontiguousarray(
            aot.reshape(QH, 128, RANK).transpose(1, 0, 2).reshape(128, QH * RANK)
        ).astype(BF16)
        m["bo_t"] = np.ascontiguousarray(bo[qs].T).astype(BF16)
        in_maps.append(m)
    return in_maps


def kernel(**inputs):
    _install_hooks()
    from concourse.bass_utils import run_bass_kernel_spmd

    if "nc" not in _CACHE:
        _CACHE["nc"] = _build()
    nc = _CACHE["nc"]

    in_maps = _host_prep(inputs)
    try:
        res = run_bass_kernel_spmd(
            nc, in_maps, core_ids=list(range(NC_)), trace=False
        )
    except Exception:
        # transient device-unrecoverable errors have been observed on the
        # first execution after a fresh compile; one retry clears them.
        res = run_bass_kernel_spmd(
            nc, in_maps, core_ids=list(range(NC_)), trace=False
        )
    out = np.empty((1, SEQ, DIM), np.float32)
    for i in range(NC_):
        out[0, :, QO * i : QO * (i + 1)] = res.results[i]["out"].T
    return out


# revision 30
# speedup vs baseline: 1.0387x; 1.0122x over previous
"""Distributed GQA attention + LoRA kernel for one TRN2 chip (8 NeuronCores).

Sharding (tensor-parallel over heads):
  core i owns Q heads 4i..4i+3 and KV head i. wq/wk/wv (+ LoRA B) are sharded
  column-wise over heads; attention is head-local. The output projection is
  sharded over the OUTPUT feature dim d (wo rows): each core computes
  out[:, 512i:512(i+1)] from the full attention output, obtained with one
  AllGather. LoRA-o's rank-16 bottleneck contracts over all heads, so each
  core ships its rank-space partial in the same AllGather and the partials
  are summed on-chip with a selection-matrix matmul.

All activations are kept transposed ([feature, seq]) so every contraction
sits on the partition axis. Weights and x are pre-transposed/cast to bf16 on
the host. RoPE uses a host-side even/odd row permutation of wq/wk so the
rotation becomes partition-half arithmetic. Attention computes S^T = K @ Q^T
directly (no P transposes); softmax is max-free exp (logits are O(10)),
row sums come from a ones-vector matmul over P^T, and 1/sum is applied while
copying the PV result out of PSUM.
"""
import math
import sys
import types

import numpy as np
import ml_dtypes

BF16 = ml_dtypes.bfloat16

DIM = 4096
SEQ = 2048
N_HEADS = 32
N_KV = 8
HD = 128
RANK = 16
NC_ = 8
QH = N_HEADS // NC_  # 4 q heads per core
QO = QH * HD  # 512 local q rows
SCALE = 1.0 / math.sqrt(HD)
AGROWS = QO + RANK  # 528 rows per rank in the all-gather payload

_CACHE = {}
DEBUG_TAPS = False


def _install_hooks():
    if "antenv.axon_hooks" in sys.modules:
        return
    mod = types.ModuleType("antenv.axon_hooks")
    mod._hook = None
    mod.set_axon_ntff_profile_hook = lambda h: setattr(mod, "_hook", h)
    mod.get_axon_ntff_profile_hook = lambda: mod._hook
    sys.modules["antenv.axon_hooks"] = mod
    try:
        from trn_agent_boot.trn_boot import _ntff_profile_via_ctypes

        mod.set_axon_ntff_profile_hook(
            _ntff_profile_via_ctypes("/opt/axon/libaxon_pjrt.so")
        )
    except Exception:
        pass
    from concourse import bass_utils

    bass_utils.upload_artifacts = lambda tmpdir: f"local://{tmpdir}"


def _split_excess_waits(nc, max_waits=1):
    """This walrus build rejects >1 sync wait on CTRL-encoded instructions.
    Move excess waits onto preceding EventSemaphore insts on the same queue."""
    import bass_rust

    n_split = 0
    for f in nc.m.functions:
        for bb in f.blocks:
            new_insts = []
            changed = False
            for ins in bb.instructions:
                si = ins.sync_info
                if si is not None and si.on_wait and len(si.on_wait) > max_waits:
                    waits = list(si.on_wait)
                    excess, keep = waits[:-max_waits], waits[-max_waits:]
                    for i in range(0, len(excess), max_waits):
                        ev = bass_rust.InstEventSemaphore(name=f"WSPLIT-{n_split}")
                        n_split += 1
                        ev.engine = ins.engine
                        ev.sync_info = bass_rust.SyncInfo(
                            on_wait=excess[i : i + max_waits], on_update=[]
                        )
                        new_insts.append(ev)
                    si.on_wait = keep
                    changed = True
                new_insts.append(ins)
            if changed:
                bb.instructions = new_insts
    return n_split


def _build():
    import concourse.bass as bass
    import concourse.mybir as mybir
    import concourse.tile as tile

    f32 = mybir.dt.float32
    bf16 = mybir.dt.bfloat16
    Alu = mybir.AluOpType
    Act = mybir.ActivationFunctionType

    nc = bass.Bass()

    def din(name, shape, dt=bf16):
        return nc.declare_dram_parameter(name, list(shape), dt, isOutput=False)

    xt = din("xt", [DIM, SEQ])
    wq_t = din("wq_t", [DIM, QO])
    wk_t = din("wk_t", [DIM, HD])
    wv_t = din("wv_t", [DIM, HD])
    aqkv_t = din("aqkv_t", [DIM, 96])
    bq_t = din("bq_t", [RANK, QO])
    bk_t = din("bk_t", [RANK, HD])
    bv_t = din("bv_t", [RANK, HD])
    wo_t = din("wo_t", [DIM, QO])
    ao_loc = din("ao_loc", [128, QH * RANK])
    bo_t = din("bo_t", [RANK, QO])
    cos_d = din("cos_d", [128, SEQ], f32)
    sin_d = din("sin_d", [128, SEQ], f32)
    maskt = din("maskt", [128, 4 * 512])
    ident = din("ident", [128, 128])
    ones_c = din("ones_c", [128, 128])
    sum_sel = din("sum_sel", [128, RANK])

    out_d = nc.declare_dram_parameter("out", [QO, SEQ], f32, isOutput=True)

    NSC = SEQ // 512  # 4 seq chunks
    NDT = DIM // 128  # 32 contraction tiles
    NKT = SEQ // 128  # 16 k tiles

    agin = [nc.dram_tensor(f"agin{c}", [AGROWS, 512], bf16) for c in range(NSC)]
    agout = [
        nc.dram_tensor(f"agout{c}", [NC_ * AGROWS, 512], bf16, addr_space="Shared")
        for c in range(NSC)
    ]

    with tile.TileContext(nc) as tc:
        with (
            tc.tile_pool(name="wpool", bufs=1) as wp,
            tc.tile_pool(name="act", bufs=1) as ap_,
            tc.tile_pool(name="stream", bufs=3) as sp,
            tc.tile_pool(name="ps", bufs=1, space="PSUM") as ps,
        ):
            # resident weights, one tile per 128-row contraction block so the
            # first matmul only waits on its own 128KB DMA. wq and wo share
            # SBUF slots (tag bigw{dt}): wo's load waits for phase A's last
            # wq read of that block.
            NG = NDT // 4  # 8 groups of 4 contraction tiles

            def load_grouped(dram, width, tagp):
                tiles = []
                for g in range(NG):
                    t = wp.tile(
                        [128, 4 * width], bf16, tag=f"{tagp}{g}", name=f"{tagp}{g}"
                    )
                    nc.sync.dma_start(
                        t.rearrange("p (t m) -> p t m", t=4),
                        dram[g * 512 : (g + 1) * 512, :].rearrange(
                            "(t p) m -> p t m", p=128
                        ),
                    )
                    tiles.append(t)
                return tiles

            wq_g = load_grouped(wq_t, QO, "bigw")
            wk_g = load_grouped(wk_t, HD, "wk")
            wv_g = load_grouped(wv_t, HD, "wv")
            aq_g = load_grouped(aqkv_t, 96, "aq")
            wq_sb = [wq_g[dt // 4][:, (dt % 4) * QO : (dt % 4 + 1) * QO] for dt in range(NDT)]
            wk_sb = [wk_g[dt // 4][:, (dt % 4) * HD : (dt % 4 + 1) * HD] for dt in range(NDT)]
            wv_sb = [wv_g[dt // 4][:, (dt % 4) * HD : (dt % 4 + 1) * HD] for dt in range(NDT)]
            aq_sb = [aq_g[dt // 4][:, (dt % 4) * 96 : (dt % 4 + 1) * 96] for dt in range(NDT)]
            bq_sb = wp.tile([RANK, QO], bf16)
            nc.sync.dma_start(bq_sb[:], bq_t[:])
            bk_sb = wp.tile([RANK, HD], bf16)
            nc.sync.dma_start(bk_sb[:], bk_t[:])
            bv_sb = wp.tile([RANK, HD], bf16)
            nc.sync.dma_start(bv_sb[:], bv_t[:])
            ao_sb = wp.tile([128, QH * RANK], bf16)
            nc.sync.dma_start(ao_sb[:], ao_loc[:])
            cos_sb = wp.tile([128, SEQ], f32)
            nc.sync.dma_start(cos_sb[:], cos_d[:])
            sin_sb = wp.tile([128, SEQ], f32)
            nc.sync.dma_start(sin_sb[:], sin_d[:])
            mask_sb = wp.tile([128, 4 * 512], bf16)
            nc.sync.dma_start(mask_sb[:], maskt[:])
            ones_sb = wp.tile([128, 128], bf16)
            nc.sync.dma_start(ones_sb[:], ones_c[:])
            bo_sb = wp.tile([RANK, QO], bf16)
            nc.sync.dma_start(bo_sb[:], bo_t[:])
            ss_sb = wp.tile([128, RANK], bf16)
            nc.sync.dma_start(ss_sb[:], sum_sel[:])

            # activations
            qT = [ap_.tile([128, SEQ], bf16, tag=f"qT{j}", name=f"qT{j}") for j in range(QH)]
            kT = ap_.tile([128, SEQ], bf16, tag="kT")
            v_nat = ap_.tile([128, NKT * 128], bf16, tag="vnat")
            attnT = [
                [
                    ap_.tile([128, 512], bf16, tag=f"attnT{j}_{c}", name=f"attnT{j}_{c}")
                    for c in range(NSC)
                ]
                for j in range(QH)
            ]

            # ---- phase A: QKV projection + LoRA (one chunk), no rope ----
            def phase_a_mm(sc):
                s0 = sc * 512
                q_ps = [ps.tile([128, 512], f32, tag=f"q{j}", name=f"q_ps{j}") for j in range(QH)]
                k_ps = ps.tile([128, 512], f32, tag="pk")
                v_ps = ps.tile([128, 512], f32, tag="pv")
                r_ps = ps.tile([96, 512], f32, tag="pr")
                xtg = [None] * NG
                for g in range(NG):
                    xtg[g] = sp.tile([128, 4 * 512], bf16, tag="xt", name="xtg", bufs=3)
                    xt_dma = nc.sync.dma_start(
                        xtg[g].rearrange("p (t m) -> p t m", t=4),
                        xt[g * 512 : (g + 1) * 512, s0 : s0 + 512].rearrange(
                            "(t p) m -> p t m", p=128
                        ),
                    )

                for dt in range(NDT):
                    xtt = xtg[dt // 4][:, (dt % 4) * 512 : (dt % 4 + 1) * 512]
                    st = dt == 0
                    # r first so its PSUM closes early for the stage-2 copies
                    nc.tensor.matmul(
                        r_ps[:], aq_sb[dt], xtt,
                        start=st, stop=(dt == NDT - 1),
                    )
                    nc.tensor.matmul(
                        k_ps[:], wk_sb[dt], xtt, start=st, stop=False,
                    )
                    nc.tensor.matmul(
                        v_ps[:], wv_sb[dt], xtt, start=st, stop=False,
                    )
                    for j in range(QH):
                        nc.tensor.matmul(
                            q_ps[j][:],
                            wq_sb[dt][:, j * 128 : (j + 1) * 128],
                            xtt,
                            start=st,
                            stop=False,
                        )
                rq = sp.tile([RANK, 512], bf16, tag="rq")
                rk = sp.tile([RANK, 512], bf16, tag="rk")
                rv = sp.tile([RANK, 512], bf16, tag="rv")
                nc.vector.tensor_copy(rq[:], r_ps[0:RANK, :])
                nc.vector.tensor_copy(rk[:], r_ps[32 : 32 + RANK, :])
                nc.vector.tensor_copy(rv[:], r_ps[64 : 64 + RANK, :])
                nc.tensor.matmul(k_ps[:], bk_sb[:], rk[:], start=False, stop=True)
                nc.tensor.matmul(v_ps[:], bv_sb[:], rv[:], start=False, stop=True)
                for j in range(QH):
                    nc.tensor.matmul(
                        q_ps[j][:], bq_sb[:, j * 128 : (j + 1) * 128], rq[:],
                        start=False, stop=True,
                    )
                # v: copy out + transpose to natural via XBAR DMA
                vTc = sp.tile([128, 512], bf16, tag="vT", bufs=2)
                nc.vector.tensor_copy(vTc[:], v_ps[:])
                for kk in range(4):
                    kt = sc * 4 + kk
                    nc.scalar.dma_start_transpose(
                        out=v_nat[:, kt * 128 : (kt + 1) * 128],
                        in_=vTc[:, kk * 128 : (kk + 1) * 128],
                    )
                return q_ps, k_ps

            def rope(src_ps, dst, s0):
                qcp = sp.tile([128, 512], f32, tag="qcp", bufs=2)
                nc.vector.tensor_copy(qcp[:], src_ps[:])
                mc = sp.tile([128, 512], f32, tag="mc", bufs=2)
                nc.vector.tensor_tensor(
                    out=mc[:], in0=qcp[:], in1=cos_sb[:, s0 : s0 + 512], op=Alu.mult
                )
                msh = sp.tile([128, 512], f32, tag="msh", bufs=2)
                nc.vector.tensor_tensor(
                    out=msh[0:64, :], in0=qcp[64:128, :],
                    in1=sin_sb[64:128, s0 : s0 + 512], op=Alu.mult,
                )
                nc.vector.tensor_tensor(
                    out=msh[64:128, :], in0=qcp[0:64, :],
                    in1=sin_sb[0:64, s0 : s0 + 512], op=Alu.mult,
                )
                nc.vector.tensor_tensor(
                    out=dst[0:64, s0 : s0 + 512], in0=mc[0:64, :],
                    in1=msh[0:64, :], op=Alu.subtract,
                )
                nc.vector.tensor_tensor(
                    out=dst[64:128, s0 : s0 + 512], in0=msh[64:128, :],
                    in1=mc[64:128, :], op=Alu.add,
                )

            wo_sb = []

            def phase_b_head(qc, h, pt):
                q0 = qc * 512
                nkt = 4 * qc + 4
                sum_ps = ps.tile([128, 512], f32, tag="q2", name="sum_ps")
                attn_ps = ps.tile(
                    [128, 512], f32, tag=("q3" if h % 2 == 0 else "c8"), name="attn_ps"
                )

                st_i = [0]

                def emit_s(kt):
                    st_tags = ("q0", "q1", "pk")
                    st_ps = ps.tile(
                        [128, 512], f32, tag=st_tags[st_i[0] % 3], name="st_ps"
                    )
                    st_i[0] += 1
                    nc.tensor.matmul(
                        st_ps[:],
                        kT[:, kt * 128 : (kt + 1) * 128],
                        qT[h][:, q0 : q0 + 512],
                        start=True,
                        stop=True,
                    )
                    jrel = kt - 4 * qc
                    if jrel >= 0:
                        nc.vector.tensor_tensor(
                            out=st_ps[:], in0=st_ps[:],
                            in1=mask_sb[:, jrel * 512 : (jrel + 1) * 512],
                            op=Alu.add,
                        )
                    nc.scalar.activation(
                        pt[:, kt * 512 : (kt + 1) * 512], st_ps[:],
                        Act.Exp, scale=SCALE,
                    )

                korder = list(range(4 * qc, nkt)) + list(range(0, 4 * qc))

                def emit_pv(kt):
                    first = kt == korder[0]
                    last = kt == korder[-1]
                    nc.tensor.matmul(
                        sum_ps[:], ones_sb[:], pt[:, kt * 512 : (kt + 1) * 512],
                        start=first, stop=last,
                    )
                    nc.tensor.matmul(
                        attn_ps[:], v_nat[:, kt * 128 : (kt + 1) * 128],
                        pt[:, kt * 512 : (kt + 1) * 512],
                        start=first, stop=last,
                    )

                # diagonal (masked) tiles first so their longer mask+exp chain
                # is covered by later S work; PV trails S by 2.
                lag = min(3, nkt - 1)
                for i, kt in enumerate(korder):
                    emit_s(kt)
                    if i >= lag:
                        emit_pv(korder[i - lag])
                for i in range(nkt - lag, nkt):
                    emit_pv(korder[i])
                invb = sp.tile([128, 512], f32, tag="invb", bufs=2)
                # reciprocal on the idle Scalar engine (ACT LUT, ~2^-10 rel —
                # ample for a bf16 result); bass's activation() refuses
                # Reciprocal so emit a Copy and patch the func.
                _i = nc.scalar.activation(
                    invb[:], sum_ps[:], Act.Copy
                )
                _i.ins.func = Act.Reciprocal
                nc.vector.tensor_tensor(
                    out=attnT[h][qc][:], in0=attn_ps[:],
                    in1=invb[:], op=Alu.mult,
                )

            ro_locs = {}

            def phase_b_ro(qc):
                ro_ps = ps.tile([RANK, 512], f32, tag="q0", name="ro_ps")
                for j in range(QH):
                    nc.tensor.matmul(
                        ro_ps[:], ao_sb[:, j * RANK : (j + 1) * RANK],
                        attnT[j][qc][:],
                        start=(j == 0), stop=(j == QH - 1),
                    )
                ro_loc = sp.tile([RANK, 512], bf16, tag="roloc", bufs=2)
                nc.vector.tensor_copy(ro_loc[:], ro_ps[:])
                ro_locs[qc] = ro_loc

            def phase_ag(qc):
                for j in range(QH):
                    nc.scalar.dma_start(
                        agin[qc][j * 128 : (j + 1) * 128, :],
                        attnT[j][qc][:],
                    )
                nc.scalar.dma_start(agin[qc][QO:AGROWS, :], ro_locs.pop(qc)[:])
                nc.gpsimd.collective_compute(
                    "AllGather",
                    mybir.AluOpType.bypass,
                    replica_groups=[list(range(NC_))],
                    ins=[agin[qc].ap().opt()],
                    outs=[agout[qc].ap().opt()],
                )

            def phase_c(qc):
                q0 = qc * 512
                # alternate PSUM tag sets so consecutive chunks double-buffer
                otags = ("pk", "pv", "pr", "c8") if qc % 2 == 0 else ("q0", "q1", "q2", "q3")
                rsum_t = sp.tile([128, 512], bf16, tag="rsum", bufs=2)
                for rr in range(NC_):
                    nc.scalar.dma_start(
                        rsum_t[rr * RANK : (rr + 1) * RANK, :],
                        agout[qc][rr * AGROWS + QO : (rr + 1) * AGROWS, :],
                    )
                ro_ps2 = ps.tile([RANK, 512], f32, tag=otags[0], name="ro_ps2")
                nc.tensor.matmul(ro_ps2[:], ss_sb[:], rsum_t[:], start=True, stop=True)
                ro2 = sp.tile([RANK, 512], bf16, tag="ro2sb", bufs=2)
                nc.vector.tensor_copy(ro2[:], ro_ps2[:])
                out_ps = [
                    ps.tile([128, 512], f32, tag=t, name=f"out_ps_{t}")
                    for t in otags
                ]
                for r in range(NC_):
                    attg = sp.tile([128, 4 * 512], bf16, tag="att", bufs=4)
                    nc.sync.dma_start(
                        attg.rearrange("p (t m) -> p t m", t=4),
                        agout[qc][r * AGROWS : r * AGROWS + QO, :].rearrange(
                            "(t p) m -> p t m", p=128
                        ),
                    )
                    for j2 in range(4):
                        ot = r * 4 + j2
                        att = attg[:, j2 * 512 : (j2 + 1) * 512]
                        for j in range(4):
                            nc.tensor.matmul(
                                out_ps[j][:],
                                wo_sb[ot][:, j * 128 : (j + 1) * 128],
                                att,
                                start=(ot == 0),
                                stop=False,
                            )
                for j in range(4):
                    nc.tensor.matmul(
                        out_ps[j][:], bo_sb[:, j * 128 : (j + 1) * 128], ro2[:],
                        start=False, stop=True,
                    )
                for j in range(4):
                    ot_sb = sp.tile([128, 512], f32, tag="osb", bufs=1)
                    nc.vector.tensor_copy(ot_sb[:], out_ps[j][:])
                    nc.scalar.dma_start(
                        out_d[j * 128 : (j + 1) * 128, q0 : q0 + 512], ot_sb[:]
                    )

            # schedule: fine-grained A/B interleave; each head's attention is
            # emitted right after its q tile's rope so the PE never waits for
            # the whole rope burst. AGs fire per chunk; C runs contiguously.
            for sc in range(NSC):
                s0 = sc * 512
                q_ps, k_ps = phase_a_mm(sc)
                if sc == NSC - 1:
                    wo_g = load_grouped(wo_t, QO, "bigw")
                    wo_sb.extend(
                        wo_g[dt // 4][:, (dt % 4) * QO : (dt % 4 + 1) * QO]
                        for dt in range(NDT)
                    )
                rope(k_ps, kT, s0)
                pt = ap_.tile([128, NKT * 512], bf16, tag="pt", name="pt", bufs=1)
                for h in range(QH):
                    rope(q_ps[h], qT[h], s0)
                    phase_b_head(sc, h, pt)
                    if h == 0 and sc >= 1:
                        # previous chunk's AllGather fires here so its HBM
                        # burst lands in this PE-bound region, not on the
                        # next chunk's xt stream.
                        phase_ag(sc - 1)
                phase_b_ro(sc)
            phase_ag(NSC - 1)
            for qc in range(NSC):
                phase_c(qc)

    _split_excess_waits(nc)
    return nc


def _host_prep(inputs):
    x = np.asarray(inputs["x"], np.float32)[0]  # [SEQ, DIM]
    wq = np.asarray(inputs["wq"], np.float32)
    wk = np.asarray(inputs["wk"], np.float32)
    wv = np.asarray(inputs["wv"], np.float32)
    wo = np.asarray(inputs["wo"], np.float32)
    fc = np.asarray(inputs["freqs_cos"], np.float32)  # [SEQ, 64]
    fs = np.asarray(inputs["freqs_sin"], np.float32)
    aq = np.asarray(inputs["lora_q_A"], np.float32)
    bq = np.asarray(inputs["lora_q_B"], np.float32)
    ak = np.asarray(inputs["lora_k_A"], np.float32)
    bk = np.asarray(inputs["lora_k_B"], np.float32)
    av = np.asarray(inputs["lora_v_A"], np.float32)
    bv = np.asarray(inputs["lora_v_B"], np.float32)
    ao = np.asarray(inputs["lora_o_A"], np.float32)
    bo = np.asarray(inputs["lora_o_B"], np.float32)

    # even/odd permutation inside each head block (RoPE layout)
    def perm(n_heads):
        p = []
        for h in range(n_heads):
            p.extend(h * HD + np.r_[0:HD:2])
            p.extend(h * HD + np.r_[1:HD:2])
        return np.array(p)

    pq, pk = perm(N_HEADS), perm(N_KV)
    wq_p, wk_p = wq[pq], wk[pk]
    bq_p, bk_p = bq[pq], bk[pk]

    xt = np.ascontiguousarray(x.T).astype(BF16)  # [DIM, SEQ]
    cos_t = np.ascontiguousarray(fc.T)  # [64, SEQ]
    sin_t = np.ascontiguousarray(fs.T)
    cos_d = np.concatenate([cos_t, cos_t], 0).astype(np.float32)
    sin_d = np.concatenate([sin_t, sin_t], 0).astype(np.float32)

    # mask tiles for the S^T diagonal chunks: [128 k, 4*512] f32
    kp = np.arange(128)[:, None]
    qp = np.arange(128)[None, :]
    tri = np.where(kp > qp, np.float32(-1e9), np.float32(0.0))  # [128k,128q]
    maskt = np.zeros((128, 4 * 512), np.float32)
    for jrel in range(4):
        blk = np.zeros((128, 512), np.float32)
        for jj in range(4):
            if jj < jrel:
                blk[:, jj * 128 : (jj + 1) * 128] = -1e9
            elif jj == jrel:
                blk[:, jj * 128 : (jj + 1) * 128] = tri
        maskt[:, jrel * 512 : (jrel + 1) * 512] = blk

    ident = np.eye(128, dtype=BF16)
    ones_c = np.ones((128, 128), BF16)
    sum_sel = np.zeros((128, RANK), np.float32)
    for r in range(NC_):
        for p in range(RANK):
            sum_sel[r * RANK + p, p] = 1.0
    sum_sel = sum_sel.astype(BF16)

    aqkv = np.zeros((96, DIM), np.float32)
    aqkv[0:RANK] = aq
    aqkv[32 : 32 + RANK] = ak
    aqkv[64 : 64 + RANK] = av
    aqkv_t = np.ascontiguousarray(aqkv.T).astype(BF16)

    shared = dict(
        xt=xt,
        aqkv_t=aqkv_t,
        cos_d=cos_d,
        sin_d=sin_d,
        maskt=maskt.astype(BF16),
        ident=ident,
        ones_c=ones_c,
        sum_sel=sum_sel,
    )
    in_maps = []
    for i in range(NC_):
        qs = slice(QO * i, QO * (i + 1))
        ks = slice(HD * i, HD * (i + 1))
        m = dict(shared)
        m["wq_t"] = np.ascontiguousarray(wq_p[qs].T).astype(BF16)
        m["wk_t"] = np.ascontiguousarray(wk_p[ks].T).astype(BF16)
        m["wv_t"] = np.ascontiguousarray(wv[ks].T).astype(BF16)
        m["bq_t"] = np.ascontiguousarray(bq_p[qs].T).astype(BF16)
        m["bk_t"] = np.ascontiguousarray(bk_p[ks].T).astype(BF16)
        m["bv_t"] = np.ascontiguousarray(bv[ks].T).astype(BF16)
        m["wo_t"] = np.ascontiguousarray(wo[qs].T).astype(BF16)
        # A_o^T rows for this core's local heads, laid out per o-tile
        aot = np.ascontiguousarray(ao.T[qs])  # [512, 16]
        m["ao_loc"] = np.ascontiguousarray(
            aot.reshape(QH, 128, RANK).transpose(1, 0, 2).reshape(128, QH * RANK)
        ).astype(BF16)
        m["bo_t"] = np.ascontiguousarray(bo[qs].T).astype(BF16)
        in_maps.append(m)
    return in_maps


def kernel(**inputs):
    _install_hooks()
    from concourse.bass_utils import run_bass_kernel_spmd

    if "nc" not in _CACHE:
        _CACHE["nc"] = _build()
    nc = _CACHE["nc"]

    in_maps = _host_prep(inputs)
    try:
        res = run_bass_kernel_spmd(
            nc, in_maps, core_ids=list(range(NC_)), trace=False
        )
    except Exception:
        # transient device-unrecoverable errors have been observed on the
        # first execution after a fresh compile; one retry clears them.
        res = run_bass_kernel_spmd(
            nc, in_maps, core_ids=list(range(NC_)), trace=False
        )
    out = np.empty((1, SEQ, DIM), np.float32)
    for i in range(NC_):
        out[0, :, QO * i : QO * (i + 1)] = res.results[i]["out"].T
    return out


# revision 31
# speedup vs baseline: 1.0696x; 1.0298x over previous
"""Distributed GQA attention + LoRA kernel for one TRN2 chip (8 NeuronCores).

Sharding (tensor-parallel over heads):
  core i owns Q heads 4i..4i+3 and KV head i. wq/wk/wv (+ LoRA B) are sharded
  column-wise over heads; attention is head-local. The output projection is
  sharded over the OUTPUT feature dim d (wo rows): each core computes
  out[:, 512i:512(i+1)] from the full attention output, obtained with one
  AllGather. LoRA-o's rank-16 bottleneck contracts over all heads, so each
  core ships its rank-space partial in the same AllGather and the partials
  are summed on-chip with a selection-matrix matmul.

All activations are kept transposed ([feature, seq]) so every contraction
sits on the partition axis. Weights and x are pre-transposed/cast to bf16 on
the host. RoPE uses a host-side even/odd row permutation of wq/wk so the
rotation becomes partition-half arithmetic. Attention computes S^T = K @ Q^T
directly (no P transposes); softmax is max-free exp (logits are O(10)),
row sums come from a ones-vector matmul over P^T, and 1/sum is applied while
copying the PV result out of PSUM.
"""
import math
import sys
import types

import numpy as np
import ml_dtypes

BF16 = ml_dtypes.bfloat16

DIM = 4096
SEQ = 2048
N_HEADS = 32
N_KV = 8
HD = 128
RANK = 16
NC_ = 8
QH = N_HEADS // NC_  # 4 q heads per core
QO = QH * HD  # 512 local q rows
SCALE = 1.0 / math.sqrt(HD)
AGROWS = QO + RANK  # 528 rows per rank in the all-gather payload

_CACHE = {}
DEBUG_TAPS = False


def _install_hooks():
    if "antenv.axon_hooks" in sys.modules:
        return
    mod = types.ModuleType("antenv.axon_hooks")
    mod._hook = None
    mod.set_axon_ntff_profile_hook = lambda h: setattr(mod, "_hook", h)
    mod.get_axon_ntff_profile_hook = lambda: mod._hook
    sys.modules["antenv.axon_hooks"] = mod
    try:
        from trn_agent_boot.trn_boot import _ntff_profile_via_ctypes

        mod.set_axon_ntff_profile_hook(
            _ntff_profile_via_ctypes("/opt/axon/libaxon_pjrt.so")
        )
    except Exception:
        pass
    from concourse import bass_utils

    bass_utils.upload_artifacts = lambda tmpdir: f"local://{tmpdir}"


def _split_excess_waits(nc, max_waits=1):
    """This walrus build rejects >1 sync wait on CTRL-encoded instructions.
    Move excess waits onto preceding EventSemaphore insts on the same queue."""
    import bass_rust

    n_split = 0
    for f in nc.m.functions:
        for bb in f.blocks:
            new_insts = []
            changed = False
            for ins in bb.instructions:
                si = ins.sync_info
                if si is not None and si.on_wait and len(si.on_wait) > max_waits:
                    waits = list(si.on_wait)
                    excess, keep = waits[:-max_waits], waits[-max_waits:]
                    for i in range(0, len(excess), max_waits):
                        ev = bass_rust.InstEventSemaphore(name=f"WSPLIT-{n_split}")
                        n_split += 1
                        ev.engine = ins.engine
                        ev.sync_info = bass_rust.SyncInfo(
                            on_wait=excess[i : i + max_waits], on_update=[]
                        )
                        new_insts.append(ev)
                    si.on_wait = keep
                    changed = True
                new_insts.append(ins)
            if changed:
                bb.instructions = new_insts
    return n_split


def _build():
    import concourse.bass as bass
    import concourse.mybir as mybir
    import concourse.tile as tile

    f32 = mybir.dt.float32
    bf16 = mybir.dt.bfloat16
    Alu = mybir.AluOpType
    Act = mybir.ActivationFunctionType

    nc = bass.Bass()

    def din(name, shape, dt=bf16):
        return nc.declare_dram_parameter(name, list(shape), dt, isOutput=False)

    xt = din("xt", [DIM, SEQ])
    wq_t = din("wq_t", [DIM, QO])
    wk_t = din("wk_t", [DIM, HD])
    wv_t = din("wv_t", [DIM, HD])
    aqkv_t = din("aqkv_t", [DIM, 96])
    bq_t = din("bq_t", [RANK, QO])
    bk_t = din("bk_t", [RANK, HD])
    bv_t = din("bv_t", [RANK, HD])
    wo_t = din("wo_t", [DIM, QO])
    ao_loc = din("ao_loc", [128, QH * RANK])
    bo_t = din("bo_t", [RANK, QO])
    cos_d = din("cos_d", [128, SEQ], f32)
    sin_d = din("sin_d", [128, SEQ], f32)
    maskt = din("maskt", [128, 4 * 512])
    ident = din("ident", [128, 128])
    ones_c = din("ones_c", [128, 128])
    sum_sel = din("sum_sel", [128, RANK])

    out_d = nc.declare_dram_parameter("out", [QO, SEQ], f32, isOutput=True)

    NSC = SEQ // 512  # 4 seq chunks
    NDT = DIM // 128  # 32 contraction tiles
    NKT = SEQ // 128  # 16 k tiles

    agin = [nc.dram_tensor(f"agin{c}", [AGROWS, 512], bf16) for c in range(NSC)]
    agout = [
        nc.dram_tensor(f"agout{c}", [NC_ * AGROWS, 512], bf16, addr_space="Shared")
        for c in range(NSC)
    ]

    with tile.TileContext(nc) as tc:
        with (
            tc.tile_pool(name="wpool", bufs=1) as wp,
            tc.tile_pool(name="act", bufs=1) as ap_,
            tc.tile_pool(name="stream", bufs=3) as sp,
            tc.tile_pool(name="ps", bufs=1, space="PSUM") as ps,
        ):
            # resident weights, one tile per 128-row contraction block so the
            # first matmul only waits on its own 128KB DMA. wq and wo share
            # SBUF slots (tag bigw{dt}): wo's load waits for phase A's last
            # wq read of that block.
            NG = NDT // 4  # 8 groups of 4 contraction tiles

            def load_grouped(dram, width, tagp):
                tiles = []
                for g in range(NG):
                    t = wp.tile(
                        [128, 4 * width], bf16, tag=f"{tagp}{g}", name=f"{tagp}{g}"
                    )
                    nc.sync.dma_start(
                        t.rearrange("p (t m) -> p t m", t=4),
                        dram[g * 512 : (g + 1) * 512, :].rearrange(
                            "(t p) m -> p t m", p=128
                        ),
                    )
                    tiles.append(t)
                return tiles

            wq_g = load_grouped(wq_t, QO, "bigw")
            wk_g = load_grouped(wk_t, HD, "wk")
            wv_g = load_grouped(wv_t, HD, "wv")
            aq_g = load_grouped(aqkv_t, 96, "aq")
            wq_sb = [wq_g[dt // 4][:, (dt % 4) * QO : (dt % 4 + 1) * QO] for dt in range(NDT)]
            wk_sb = [wk_g[dt // 4][:, (dt % 4) * HD : (dt % 4 + 1) * HD] for dt in range(NDT)]
            wv_sb = [wv_g[dt // 4][:, (dt % 4) * HD : (dt % 4 + 1) * HD] for dt in range(NDT)]
            aq_sb = [aq_g[dt // 4][:, (dt % 4) * 96 : (dt % 4 + 1) * 96] for dt in range(NDT)]
            bq_sb = wp.tile([RANK, QO], bf16)
            nc.sync.dma_start(bq_sb[:], bq_t[:])
            bk_sb = wp.tile([RANK, HD], bf16)
            nc.sync.dma_start(bk_sb[:], bk_t[:])
            bv_sb = wp.tile([RANK, HD], bf16)
            nc.sync.dma_start(bv_sb[:], bv_t[:])
            ao_sb = wp.tile([128, QH * RANK], bf16)
            nc.sync.dma_start(ao_sb[:], ao_loc[:])
            cos_sb = wp.tile([128, SEQ], f32)
            nc.sync.dma_start(cos_sb[:], cos_d[:])
            sin_sb = wp.tile([128, SEQ], f32)
            nc.sync.dma_start(sin_sb[:], sin_d[:])
            mask_sb = wp.tile([128, 4 * 512], bf16)
            nc.sync.dma_start(mask_sb[:], maskt[:])
            ones_sb = wp.tile([128, 128], bf16)
            nc.sync.dma_start(ones_sb[:], ones_c[:])
            bo_sb = wp.tile([RANK, QO], bf16)
            nc.sync.dma_start(bo_sb[:], bo_t[:])
            ss_sb = wp.tile([128, RANK], bf16)
            nc.sync.dma_start(ss_sb[:], sum_sel[:])

            # activations
            qT = [ap_.tile([128, SEQ], bf16, tag=f"qT{j}", name=f"qT{j}") for j in range(QH)]
            kT = ap_.tile([128, SEQ], bf16, tag="kT")
            v_nat = ap_.tile([128, NKT * 128], bf16, tag="vnat")
            attnT = [
                ap_.tile([128, QH * 512], bf16, tag=f"attnT_{c}", name=f"attnT_{c}")
                for c in range(NSC)
            ]

            # ---- phase A: QKV projection + LoRA (one chunk), no rope ----
            def phase_a_mm(sc):
                s0 = sc * 512
                q_ps = [ps.tile([128, 512], f32, tag=f"q{j}", name=f"q_ps{j}") for j in range(QH)]
                k_ps = ps.tile([128, 512], f32, tag="pk")
                v_ps = ps.tile([128, 512], f32, tag="pv")
                r_ps = ps.tile([96, 512], f32, tag="pr")
                xtg = [None] * NG
                for g in range(NG):
                    xtg[g] = sp.tile([128, 4 * 512], bf16, tag="xt", name="xtg", bufs=3)
                    xt_dma = nc.sync.dma_start(
                        xtg[g].rearrange("p (t m) -> p t m", t=4),
                        xt[g * 512 : (g + 1) * 512, s0 : s0 + 512].rearrange(
                            "(t p) m -> p t m", p=128
                        ),
                    )

                for dt in range(NDT):
                    xtt = xtg[dt // 4][:, (dt % 4) * 512 : (dt % 4 + 1) * 512]
                    st = dt == 0
                    # r first so its PSUM closes early for the stage-2 copies
                    nc.tensor.matmul(
                        r_ps[:], aq_sb[dt], xtt,
                        start=st, stop=(dt == NDT - 1),
                    )
                    nc.tensor.matmul(
                        k_ps[:], wk_sb[dt], xtt, start=st, stop=False,
                    )
                    nc.tensor.matmul(
                        v_ps[:], wv_sb[dt], xtt, start=st, stop=False,
                    )
                    for j in range(QH):
                        nc.tensor.matmul(
                            q_ps[j][:],
                            wq_sb[dt][:, j * 128 : (j + 1) * 128],
                            xtt,
                            start=st,
                            stop=False,
                        )
                rq = sp.tile([RANK, 512], bf16, tag="rq")
                rk = sp.tile([RANK, 512], bf16, tag="rk")
                rv = sp.tile([RANK, 512], bf16, tag="rv")
                nc.vector.tensor_copy(rq[:], r_ps[0:RANK, :])
                nc.vector.tensor_copy(rk[:], r_ps[32 : 32 + RANK, :])
                nc.vector.tensor_copy(rv[:], r_ps[64 : 64 + RANK, :])
                nc.tensor.matmul(k_ps[:], bk_sb[:], rk[:], start=False, stop=True)
                nc.tensor.matmul(v_ps[:], bv_sb[:], rv[:], start=False, stop=True)
                for j in range(QH):
                    nc.tensor.matmul(
                        q_ps[j][:], bq_sb[:, j * 128 : (j + 1) * 128], rq[:],
                        start=False, stop=True,
                    )
                # v: copy out + transpose to natural via XBAR DMA
                vTc = sp.tile([128, 512], bf16, tag="vT", bufs=2)
                nc.vector.tensor_copy(vTc[:], v_ps[:])
                for kk in range(4):
                    kt = sc * 4 + kk
                    nc.scalar.dma_start_transpose(
                        out=v_nat[:, kt * 128 : (kt + 1) * 128],
                        in_=vTc[:, kk * 128 : (kk + 1) * 128],
                    )
                return q_ps, k_ps

            def rope(src_ps, dst, s0):
                qcp = sp.tile([128, 512], f32, tag="qcp", bufs=2)
                nc.vector.tensor_copy(qcp[:], src_ps[:])
                mc = sp.tile([128, 512], f32, tag="mc", bufs=2)
                nc.vector.tensor_tensor(
                    out=mc[:], in0=qcp[:], in1=cos_sb[:, s0 : s0 + 512], op=Alu.mult
                )
                msh = sp.tile([128, 512], f32, tag="msh", bufs=2)
                nc.vector.tensor_tensor(
                    out=msh[0:64, :], in0=qcp[64:128, :],
                    in1=sin_sb[64:128, s0 : s0 + 512], op=Alu.mult,
                )
                nc.vector.tensor_tensor(
                    out=msh[64:128, :], in0=qcp[0:64, :],
                    in1=sin_sb[0:64, s0 : s0 + 512], op=Alu.mult,
                )
                nc.vector.tensor_tensor(
                    out=dst[0:64, s0 : s0 + 512], in0=mc[0:64, :],
                    in1=msh[0:64, :], op=Alu.subtract,
                )
                nc.vector.tensor_tensor(
                    out=dst[64:128, s0 : s0 + 512], in0=msh[64:128, :],
                    in1=mc[64:128, :], op=Alu.add,
                )

            wo_sb = []

            def phase_b_head(qc, h, pt):
                q0 = qc * 512
                nkt = 4 * qc + 4
                sum_ps = ps.tile([128, 512], f32, tag="q2", name="sum_ps")
                attn_ps = ps.tile(
                    [128, 512], f32, tag=("q3" if h % 2 == 0 else "c8"), name="attn_ps"
                )

                st_i = [0]

                def emit_s(kt):
                    st_tags = ("q0", "q1", "pk")
                    st_ps = ps.tile(
                        [128, 512], f32, tag=st_tags[st_i[0] % 3], name="st_ps"
                    )
                    st_i[0] += 1
                    nc.tensor.matmul(
                        st_ps[:],
                        kT[:, kt * 128 : (kt + 1) * 128],
                        qT[h][:, q0 : q0 + 512],
                        start=True,
                        stop=True,
                    )
                    jrel = kt - 4 * qc
                    if jrel >= 0:
                        nc.vector.tensor_tensor(
                            out=st_ps[:], in0=st_ps[:],
                            in1=mask_sb[:, jrel * 512 : (jrel + 1) * 512],
                            op=Alu.add,
                        )
                    nc.scalar.activation(
                        pt[:, kt * 512 : (kt + 1) * 512], st_ps[:],
                        Act.Exp, scale=SCALE,
                    )

                diag = list(range(4 * qc, nkt))
                plain = list(range(0, 4 * qc))
                korder = []
                step = max(1, (nkt + 3) // 4)
                pi = 0
                for d in diag:
                    korder.append(d)
                    take = plain[pi : pi + step - 1]
                    korder.extend(take)
                    pi += len(take)
                korder.extend(plain[pi:])

                def emit_pv(kt):
                    first = kt == korder[0]
                    last = kt == korder[-1]
                    nc.tensor.matmul(
                        sum_ps[:], ones_sb[:], pt[:, kt * 512 : (kt + 1) * 512],
                        start=first, stop=last,
                    )
                    nc.tensor.matmul(
                        attn_ps[:], v_nat[:, kt * 128 : (kt + 1) * 128],
                        pt[:, kt * 512 : (kt + 1) * 512],
                        start=first, stop=last,
                    )

                # diagonal (masked) tiles first so their longer mask+exp chain
                # is covered by later S work; PV trails S by 2.
                lag = min(3, nkt - 1)
                for i, kt in enumerate(korder):
                    emit_s(kt)
                    if i >= lag:
                        emit_pv(korder[i - lag])
                for i in range(nkt - lag, nkt):
                    emit_pv(korder[i])
                invb = sp.tile([128, 512], f32, tag="invb", bufs=2)
                # reciprocal on the idle Scalar engine (ACT LUT, ~2^-10 rel —
                # ample for a bf16 result); bass's activation() refuses
                # Reciprocal so emit a Copy and patch the func.
                _i = nc.scalar.activation(
                    invb[:], sum_ps[:], Act.Copy
                )
                _i.ins.func = Act.Reciprocal
                nc.vector.tensor_tensor(
                    out=attnT[qc][:, h * 512 : (h + 1) * 512], in0=attn_ps[:],
                    in1=invb[:], op=Alu.mult,
                )

            ro_locs = {}

            def phase_b_ro(qc):
                ro_ps = ps.tile([RANK, 512], f32, tag="q0", name="ro_ps")
                for j in range(QH):
                    nc.tensor.matmul(
                        ro_ps[:], ao_sb[:, j * RANK : (j + 1) * RANK],
                        attnT[qc][:, j * 512 : (j + 1) * 512],
                        start=(j == 0), stop=(j == QH - 1),
                    )
                ro_loc = sp.tile([RANK, 512], bf16, tag="roloc", bufs=2)
                nc.vector.tensor_copy(ro_loc[:], ro_ps[:])
                ro_locs[qc] = ro_loc

            def phase_ag(qc):
                nc.scalar.dma_start(
                    agin[qc][0:QO, :].rearrange("(j p) m -> p j m", p=128),
                    attnT[qc].rearrange("p (j m) -> p j m", j=QH),
                )
                nc.scalar.dma_start(agin[qc][QO:AGROWS, :], ro_locs.pop(qc)[:])
                nc.gpsimd.collective_compute(
                    "AllGather",
                    mybir.AluOpType.bypass,
                    replica_groups=[list(range(NC_))],
                    ins=[agin[qc].ap().opt()],
                    outs=[agout[qc].ap().opt()],
                )

            def phase_c(qc):
                q0 = qc * 512
                # alternate PSUM tag sets so consecutive chunks double-buffer
                otags = ("pk", "pv", "pr", "c8") if qc % 2 == 0 else ("q0", "q1", "q2", "q3")
                rsum_t = sp.tile([128, 512], bf16, tag="rsum", bufs=2)
                for rr in range(NC_):
                    nc.scalar.dma_start(
                        rsum_t[rr * RANK : (rr + 1) * RANK, :],
                        agout[qc][rr * AGROWS + QO : (rr + 1) * AGROWS, :],
                    )
                ro_ps2 = ps.tile([RANK, 512], f32, tag=otags[0], name="ro_ps2")
                nc.tensor.matmul(ro_ps2[:], ss_sb[:], rsum_t[:], start=True, stop=True)
                ro2 = sp.tile([RANK, 512], bf16, tag="ro2sb", bufs=2)
                nc.vector.tensor_copy(ro2[:], ro_ps2[:])
                out_ps = [
                    ps.tile([128, 512], f32, tag=t, name=f"out_ps_{t}")
                    for t in otags
                ]
                for r in range(NC_):
                    attg = sp.tile([128, 4 * 512], bf16, tag="att", bufs=4)
                    nc.sync.dma_start(
                        attg.rearrange("p (t m) -> p t m", t=4),
                        agout[qc][r * AGROWS : r * AGROWS + QO, :].rearrange(
                            "(t p) m -> p t m", p=128
                        ),
                    )
                    for j2 in range(4):
                        ot = r * 4 + j2
                        att = attg[:, j2 * 512 : (j2 + 1) * 512]
                        for j in range(4):
                            nc.tensor.matmul(
                                out_ps[j][:],
                                wo_sb[ot][:, j * 128 : (j + 1) * 128],
                                att,
                                start=(ot == 0),
                                stop=False,
                            )
                for j in range(4):
                    nc.tensor.matmul(
                        out_ps[j][:], bo_sb[:, j * 128 : (j + 1) * 128], ro2[:],
                        start=False, stop=True,
                    )
                ot_sb = sp.tile([128, 4 * 512], f32, tag="osb", bufs=1)
                for j in range(4):
                    nc.vector.tensor_copy(
                        ot_sb[:, j * 512 : (j + 1) * 512], out_ps[j][:]
                    )
                nc.scalar.dma_start(
                    out_d[:, q0 : q0 + 512].rearrange("(j p) m -> p j m", p=128),
                    ot_sb.rearrange("p (j m) -> p j m", j=4),
                )

            # schedule: fine-grained A/B interleave; each head's attention is
            # emitted right after its q tile's rope so the PE never waits for
            # the whole rope burst. AGs fire per chunk; C runs contiguously.
            for sc in range(NSC):
                s0 = sc * 512
                q_ps, k_ps = phase_a_mm(sc)
                if sc == NSC - 1:
                    wo_g = load_grouped(wo_t, QO, "bigw")
                    wo_sb.extend(
                        wo_g[dt // 4][:, (dt % 4) * QO : (dt % 4 + 1) * QO]
                        for dt in range(NDT)
                    )
                rope(k_ps, kT, s0)
                pt = ap_.tile([128, NKT * 512], bf16, tag="pt", name="pt", bufs=1)
                for h in range(QH):
                    rope(q_ps[h], qT[h], s0)
                    phase_b_head(sc, h, pt)
                    if h == 0 and sc >= 1:
                        # previous chunk's AllGather fires here so its HBM
                        # burst lands in this PE-bound region, not on the
                        # next chunk's xt stream.
                        phase_ag(sc - 1)
                phase_b_ro(sc)
            phase_ag(NSC - 1)
            for qc in range(NSC):
                phase_c(qc)

    _split_excess_waits(nc)
    return nc


def _host_prep(inputs):
    x = np.asarray(inputs["x"], np.float32)[0]  # [SEQ, DIM]
    wq = np.asarray(inputs["wq"], np.float32)
    wk = np.asarray(inputs["wk"], np.float32)
    wv = np.asarray(inputs["wv"], np.float32)
    wo = np.asarray(inputs["wo"], np.float32)
    fc = np.asarray(inputs["freqs_cos"], np.float32)  # [SEQ, 64]
    fs = np.asarray(inputs["freqs_sin"], np.float32)
    aq = np.asarray(inputs["lora_q_A"], np.float32)
    bq = np.asarray(inputs["lora_q_B"], np.float32)
    ak = np.asarray(inputs["lora_k_A"], np.float32)
    bk = np.asarray(inputs["lora_k_B"], np.float32)
    av = np.asarray(inputs["lora_v_A"], np.float32)
    bv = np.asarray(inputs["lora_v_B"], np.float32)
    ao = np.asarray(inputs["lora_o_A"], np.float32)
    bo = np.asarray(inputs["lora_o_B"], np.float32)

    # even/odd permutation inside each head block (RoPE layout)
    def perm(n_heads):
        p = []
        for h in range(n_heads):
            p.extend(h * HD + np.r_[0:HD:2])
            p.extend(h * HD + np.r_[1:HD:2])
        return np.array(p)

    pq, pk = perm(N_HEADS), perm(N_KV)
    wq_p, wk_p = wq[pq], wk[pk]
    bq_p, bk_p = bq[pq], bk[pk]

    xt = np.ascontiguousarray(x.T).astype(BF16)  # [DIM, SEQ]
    cos_t = np.ascontiguousarray(fc.T)  # [64, SEQ]
    sin_t = np.ascontiguousarray(fs.T)
    cos_d = np.concatenate([cos_t, cos_t], 0).astype(np.float32)
    sin_d = np.concatenate([sin_t, sin_t], 0).astype(np.float32)

    # mask tiles for the S^T diagonal chunks: [128 k, 4*512] f32
    kp = np.arange(128)[:, None]
    qp = np.arange(128)[None, :]
    tri = np.where(kp > qp, np.float32(-1e9), np.float32(0.0))  # [128k,128q]
    maskt = np.zeros((128, 4 * 512), np.float32)
    for jrel in range(4):
        blk = np.zeros((128, 512), np.float32)
        for jj in range(4):
            if jj < jrel:
                blk[:, jj * 128 : (jj + 1) * 128] = -1e9
            elif jj == jrel:
                blk[:, jj * 128 : (jj + 1) * 128] = tri
        maskt[:, jrel * 512 : (jrel + 1) * 512] = blk

    ident = np.eye(128, dtype=BF16)
    ones_c = np.ones((128, 128), BF16)
    sum_sel = np.zeros((128, RANK), np.float32)
    for r in range(NC_):
        for p in range(RANK):
            sum_sel[r * RANK + p, p] = 1.0
    sum_sel = sum_sel.astype(BF16)

    aqkv = np.zeros((96, DIM), np.float32)
    aqkv[0:RANK] = aq
    aqkv[32 : 32 + RANK] = ak
    aqkv[64 : 64 + RANK] = av
    aqkv_t = np.ascontiguousarray(aqkv.T).astype(BF16)

    shared = dict(
        xt=xt,
        aqkv_t=aqkv_t,
        cos_d=cos_d,
        sin_d=sin_d,
        maskt=maskt.astype(BF16),
        ident=ident,
        ones_c=ones_c,
        sum_sel=sum_sel,
    )
    in_maps = []
    for i in range(NC_):
        qs = slice(QO * i, QO * (i + 1))
        ks = slice(HD * i, HD * (i + 1))
        m = dict(shared)
        m["wq_t"] = np.ascontiguousarray(wq_p[qs].T).astype(BF16)
        m["wk_t"] = np.ascontiguousarray(wk_p[ks].T).astype(BF16)
        m["wv_t"] = np.ascontiguousarray(wv[ks].T).astype(BF16)
        m["bq_t"] = np.ascontiguousarray(bq_p[qs].T).astype(BF16)
        m["bk_t"] = np.ascontiguousarray(bk_p[ks].T).astype(BF16)
        m["bv_t"] = np.ascontiguousarray(bv[ks].T).astype(BF16)
        m["wo_t"] = np.ascontiguousarray(wo[qs].T).astype(BF16)
        # A_o^T rows for this core's local heads, laid out per o-tile
        aot = np.ascontiguousarray(ao.T[qs])  # [512, 16]
        m["ao_loc"] = np.ascontiguousarray(
            aot.reshape(QH, 128, RANK).transpose(1, 0, 2).reshape(128, QH * RANK)
        ).astype(BF16)
        m["bo_t"] = np.ascontiguousarray(bo[qs].T).astype(BF16)
        in_maps.append(m)
    return in_maps


def kernel(**inputs):
    _install_hooks()
    from concourse.bass_utils import run_bass_kernel_spmd

    if "nc" not in _CACHE:
        _CACHE["nc"] = _build()
    nc = _CACHE["nc"]

    in_maps = _host_prep(inputs)
    try:
        res = run_bass_kernel_spmd(
            nc, in_maps, core_ids=list(range(NC_)), trace=False
        )
    except Exception:
        # transient device-unrecoverable errors have been observed on the
        # first execution after a fresh compile; one retry clears them.
        res = run_bass_kernel_spmd(
            nc, in_maps, core_ids=list(range(NC_)), trace=False
        )
    out = np.empty((1, SEQ, DIM), np.float32)
    for i in range(NC_):
        out[0, :, QO * i : QO * (i + 1)] = res.results[i]["out"].T
    return out


# revision 32
# speedup vs baseline: 1.0768x; 1.0067x over previous
"""Distributed GQA attention + LoRA kernel for one TRN2 chip (8 NeuronCores).

Sharding (tensor-parallel over heads):
  core i owns Q heads 4i..4i+3 and KV head i. wq/wk/wv (+ LoRA B) are sharded
  column-wise over heads; attention is head-local. The output projection is
  sharded over the OUTPUT feature dim d (wo rows): each core computes
  out[:, 512i:512(i+1)] from the full attention output, obtained with one
  AllGather. LoRA-o's rank-16 bottleneck contracts over all heads, so each
  core ships its rank-space partial in the same AllGather and the partials
  are summed on-chip with a selection-matrix matmul.

All activations are kept transposed ([feature, seq]) so every contraction
sits on the partition axis. Weights and x are pre-transposed/cast to bf16 on
the host. RoPE uses a host-side even/odd row permutation of wq/wk so the
rotation becomes partition-half arithmetic. Attention computes S^T = K @ Q^T
directly (no P transposes); softmax is max-free exp (logits are O(10)),
row sums come from a ones-vector matmul over P^T, and 1/sum is applied while
copying the PV result out of PSUM.
"""
import math
import sys
import types

import numpy as np
import ml_dtypes

BF16 = ml_dtypes.bfloat16

DIM = 4096
SEQ = 2048
N_HEADS = 32
N_KV = 8
HD = 128
RANK = 16
NC_ = 8
QH = N_HEADS // NC_  # 4 q heads per core
QO = QH * HD  # 512 local q rows
SCALE = 1.0 / math.sqrt(HD)
AGROWS = QO + RANK  # 528 rows per rank in the all-gather payload

_CACHE = {}
DEBUG_TAPS = False


def _install_hooks():
    if "antenv.axon_hooks" in sys.modules:
        return
    mod = types.ModuleType("antenv.axon_hooks")
    mod._hook = None
    mod.set_axon_ntff_profile_hook = lambda h: setattr(mod, "_hook", h)
    mod.get_axon_ntff_profile_hook = lambda: mod._hook
    sys.modules["antenv.axon_hooks"] = mod
    try:
        from trn_agent_boot.trn_boot import _ntff_profile_via_ctypes

        mod.set_axon_ntff_profile_hook(
            _ntff_profile_via_ctypes("/opt/axon/libaxon_pjrt.so")
        )
    except Exception:
        pass
    from concourse import bass_utils

    bass_utils.upload_artifacts = lambda tmpdir: f"local://{tmpdir}"


def _split_excess_waits(nc, max_waits=1):
    """This walrus build rejects >1 sync wait on CTRL-encoded instructions.
    Move excess waits onto preceding EventSemaphore insts on the same queue."""
    import bass_rust

    n_split = 0
    for f in nc.m.functions:
        for bb in f.blocks:
            new_insts = []
            changed = False
            for ins in bb.instructions:
                si = ins.sync_info
                if si is not None and si.on_wait and len(si.on_wait) > max_waits:
                    waits = list(si.on_wait)
                    excess, keep = waits[:-max_waits], waits[-max_waits:]
                    for i in range(0, len(excess), max_waits):
                        ev = bass_rust.InstEventSemaphore(name=f"WSPLIT-{n_split}")
                        n_split += 1
                        ev.engine = ins.engine
                        ev.sync_info = bass_rust.SyncInfo(
                            on_wait=excess[i : i + max_waits], on_update=[]
                        )
                        new_insts.append(ev)
                    si.on_wait = keep
                    changed = True
                new_insts.append(ins)
            if changed:
                bb.instructions = new_insts
    return n_split


def _build():
    import concourse.bass as bass
    import concourse.mybir as mybir
    import concourse.tile as tile

    f32 = mybir.dt.float32
    bf16 = mybir.dt.bfloat16
    Alu = mybir.AluOpType
    Act = mybir.ActivationFunctionType

    nc = bass.Bass()

    def din(name, shape, dt=bf16):
        return nc.declare_dram_parameter(name, list(shape), dt, isOutput=False)

    xt = din("xt", [DIM, SEQ])
    wq_t = din("wq_t", [DIM, QO])
    wk_t = din("wk_t", [DIM, HD])
    wv_t = din("wv_t", [DIM, HD])
    aqkv_t = din("aqkv_t", [DIM, 96])
    bq_t = din("bq_t", [RANK, QO])
    bk_t = din("bk_t", [RANK, HD])
    bv_t = din("bv_t", [RANK, HD])
    wo_t = din("wo_t", [DIM, QO])
    ao_loc = din("ao_loc", [128, QH * RANK])
    bo_t = din("bo_t", [RANK, QO])
    cos_d = din("cos_d", [128, SEQ], f32)
    sin_d = din("sin_d", [128, SEQ], f32)
    maskt = din("maskt", [128, 4 * 512])
    ident = din("ident", [128, 128])
    ones_c = din("ones_c", [128, 128])
    sum_sel = din("sum_sel", [128, RANK])

    out_d = nc.declare_dram_parameter("out", [QO, SEQ], f32, isOutput=True)

    NSC = SEQ // 512  # 4 seq chunks
    NDT = DIM // 128  # 32 contraction tiles
    NKT = SEQ // 128  # 16 k tiles

    agin = [nc.dram_tensor(f"agin{c}", [AGROWS, 512], bf16) for c in range(NSC)]
    agout = [
        nc.dram_tensor(f"agout{c}", [NC_ * AGROWS, 512], bf16, addr_space="Shared")
        for c in range(NSC)
    ]

    with tile.TileContext(nc) as tc:
        with (
            tc.tile_pool(name="wpool", bufs=1) as wp,
            tc.tile_pool(name="act", bufs=1) as ap_,
            tc.tile_pool(name="stream", bufs=3) as sp,
            tc.tile_pool(name="ps", bufs=1, space="PSUM") as ps,
        ):
            # resident weights, one tile per 128-row contraction block so the
            # first matmul only waits on its own 128KB DMA. wq and wo share
            # SBUF slots (tag bigw{dt}): wo's load waits for phase A's last
            # wq read of that block.
            NG = NDT // 4  # 8 groups of 4 contraction tiles

            def load_grouped(dram, width, tagp):
                tiles = []
                for g in range(NG):
                    t = wp.tile(
                        [128, 4 * width], bf16, tag=f"{tagp}{g}", name=f"{tagp}{g}"
                    )
                    nc.sync.dma_start(
                        t.rearrange("p (t m) -> p t m", t=4),
                        dram[g * 512 : (g + 1) * 512, :].rearrange(
                            "(t p) m -> p t m", p=128
                        ),
                    )
                    tiles.append(t)
                return tiles

            # group-major emission so the first dt-group's four weights land
            # within the first ~1MB of DMA traffic
            wq_g, wk_g, wv_g, aq_g = [], [], [], []
            for g in range(NG):
                for dram, width, tagp, lst in (
                    (aqkv_t, 96, "aq", aq_g),
                    (wk_t, HD, "wk", wk_g),
                    (wv_t, HD, "wv", wv_g),
                    (wq_t, QO, "bigw", wq_g),
                ):
                    t = wp.tile(
                        [128, 4 * width], bf16, tag=f"{tagp}{g}", name=f"{tagp}{g}"
                    )
                    nc.sync.dma_start(
                        t.rearrange("p (t m) -> p t m", t=4),
                        dram[g * 512 : (g + 1) * 512, :].rearrange(
                            "(t p) m -> p t m", p=128
                        ),
                    )
                    lst.append(t)
            wq_sb = [wq_g[dt // 4][:, (dt % 4) * QO : (dt % 4 + 1) * QO] for dt in range(NDT)]
            wk_sb = [wk_g[dt // 4][:, (dt % 4) * HD : (dt % 4 + 1) * HD] for dt in range(NDT)]
            wv_sb = [wv_g[dt // 4][:, (dt % 4) * HD : (dt % 4 + 1) * HD] for dt in range(NDT)]
            aq_sb = [aq_g[dt // 4][:, (dt % 4) * 96 : (dt % 4 + 1) * 96] for dt in range(NDT)]
            bq_sb = wp.tile([RANK, QO], bf16)
            nc.sync.dma_start(bq_sb[:], bq_t[:])
            bk_sb = wp.tile([RANK, HD], bf16)
            nc.sync.dma_start(bk_sb[:], bk_t[:])
            bv_sb = wp.tile([RANK, HD], bf16)
            nc.sync.dma_start(bv_sb[:], bv_t[:])
            ao_sb = wp.tile([128, QH * RANK], bf16)
            nc.sync.dma_start(ao_sb[:], ao_loc[:])
            cos_sb = wp.tile([128, SEQ], f32)
            nc.sync.dma_start(cos_sb[:], cos_d[:])
            sin_sb = wp.tile([128, SEQ], f32)
            nc.sync.dma_start(sin_sb[:], sin_d[:])
            mask_sb = wp.tile([128, 4 * 512], bf16)
            nc.sync.dma_start(mask_sb[:], maskt[:])
            ones_sb = wp.tile([128, 128], bf16)
            nc.sync.dma_start(ones_sb[:], ones_c[:])
            bo_sb = wp.tile([RANK, QO], bf16)
            nc.sync.dma_start(bo_sb[:], bo_t[:])
            ss_sb = wp.tile([128, RANK], bf16)
            nc.sync.dma_start(ss_sb[:], sum_sel[:])

            # activations
            qT = [ap_.tile([128, SEQ], bf16, tag=f"qT{j}", name=f"qT{j}") for j in range(QH)]
            kT = ap_.tile([128, SEQ], bf16, tag="kT")
            v_nat = ap_.tile([128, NKT * 128], bf16, tag="vnat")
            attnT = [
                ap_.tile([128, QH * 512], bf16, tag=f"attnT_{c}", name=f"attnT_{c}")
                for c in range(NSC)
            ]

            # ---- phase A: QKV projection + LoRA (one chunk), no rope ----
            def phase_a_mm(sc):
                s0 = sc * 512
                q_ps = [ps.tile([128, 512], f32, tag=f"q{j}", name=f"q_ps{j}") for j in range(QH)]
                k_ps = ps.tile([128, 512], f32, tag="pk")
                v_ps = ps.tile([128, 512], f32, tag="pv")
                r_ps = ps.tile([96, 512], f32, tag="pr")
                xtg = [None] * NG
                for g in range(NG):
                    xtg[g] = sp.tile([128, 4 * 512], bf16, tag="xt", name="xtg", bufs=3)
                    xt_dma = nc.sync.dma_start(
                        xtg[g].rearrange("p (t m) -> p t m", t=4),
                        xt[g * 512 : (g + 1) * 512, s0 : s0 + 512].rearrange(
                            "(t p) m -> p t m", p=128
                        ),
                    )

                for dt in range(NDT):
                    xtt = xtg[dt // 4][:, (dt % 4) * 512 : (dt % 4 + 1) * 512]
                    st = dt == 0
                    # r first so its PSUM closes early for the stage-2 copies
                    nc.tensor.matmul(
                        r_ps[:], aq_sb[dt], xtt,
                        start=st, stop=(dt == NDT - 1),
                    )
                    nc.tensor.matmul(
                        k_ps[:], wk_sb[dt], xtt, start=st, stop=False,
                    )
                    nc.tensor.matmul(
                        v_ps[:], wv_sb[dt], xtt, start=st, stop=False,
                    )
                    for j in range(QH):
                        nc.tensor.matmul(
                            q_ps[j][:],
                            wq_sb[dt][:, j * 128 : (j + 1) * 128],
                            xtt,
                            start=st,
                            stop=False,
                        )
                rq = sp.tile([RANK, 512], bf16, tag="rq")
                rk = sp.tile([RANK, 512], bf16, tag="rk")
                rv = sp.tile([RANK, 512], bf16, tag="rv")
                nc.vector.tensor_copy(rq[:], r_ps[0:RANK, :])
                nc.vector.tensor_copy(rk[:], r_ps[32 : 32 + RANK, :])
                nc.vector.tensor_copy(rv[:], r_ps[64 : 64 + RANK, :])
                nc.tensor.matmul(k_ps[:], bk_sb[:], rk[:], start=False, stop=True)
                nc.tensor.matmul(v_ps[:], bv_sb[:], rv[:], start=False, stop=True)
                for j in range(QH):
                    nc.tensor.matmul(
                        q_ps[j][:], bq_sb[:, j * 128 : (j + 1) * 128], rq[:],
                        start=False, stop=True,
                    )
                # v: copy out + transpose to natural via XBAR DMA
                vTc = sp.tile([128, 512], bf16, tag="vT", bufs=2)
                nc.vector.tensor_copy(vTc[:], v_ps[:])
                for kk in range(4):
                    kt = sc * 4 + kk
                    nc.scalar.dma_start_transpose(
                        out=v_nat[:, kt * 128 : (kt + 1) * 128],
                        in_=vTc[:, kk * 128 : (kk + 1) * 128],
                    )
                return q_ps, k_ps

            def rope(src_ps, dst, s0):
                qcp = sp.tile([128, 512], f32, tag="qcp", bufs=2)
                nc.vector.tensor_copy(qcp[:], src_ps[:])
                mc = sp.tile([128, 512], f32, tag="mc", bufs=2)
                nc.vector.tensor_tensor(
                    out=mc[:], in0=qcp[:], in1=cos_sb[:, s0 : s0 + 512], op=Alu.mult
                )
                msh = sp.tile([128, 512], f32, tag="msh", bufs=2)
                nc.vector.tensor_tensor(
                    out=msh[0:64, :], in0=qcp[64:128, :],
                    in1=sin_sb[64:128, s0 : s0 + 512], op=Alu.mult,
                )
                nc.vector.tensor_tensor(
                    out=msh[64:128, :], in0=qcp[0:64, :],
                    in1=sin_sb[0:64, s0 : s0 + 512], op=Alu.mult,
                )
                nc.vector.tensor_tensor(
                    out=dst[0:64, s0 : s0 + 512], in0=mc[0:64, :],
                    in1=msh[0:64, :], op=Alu.subtract,
                )
                nc.vector.tensor_tensor(
                    out=dst[64:128, s0 : s0 + 512], in0=msh[64:128, :],
                    in1=mc[64:128, :], op=Alu.add,
                )

            wo_sb = []

            def phase_b_head(qc, h, pt):
                q0 = qc * 512
                nkt = 4 * qc + 4
                sum_ps = ps.tile([128, 512], f32, tag="q2", name="sum_ps")
                attn_ps = ps.tile(
                    [128, 512], f32, tag=("q3" if h % 2 == 0 else "c8"), name="attn_ps"
                )

                st_i = [0]

                def emit_s(kt):
                    st_tags = ("q0", "q1", "pk")
                    st_ps = ps.tile(
                        [128, 512], f32, tag=st_tags[st_i[0] % 3], name="st_ps"
                    )
                    st_i[0] += 1
                    nc.tensor.matmul(
                        st_ps[:],
                        kT[:, kt * 128 : (kt + 1) * 128],
                        qT[h][:, q0 : q0 + 512],
                        start=True,
                        stop=True,
                    )
                    jrel = kt - 4 * qc
                    if jrel >= 0:
                        nc.vector.tensor_tensor(
                            out=st_ps[:], in0=st_ps[:],
                            in1=mask_sb[:, jrel * 512 : (jrel + 1) * 512],
                            op=Alu.add,
                        )
                    nc.scalar.activation(
                        pt[:, kt * 512 : (kt + 1) * 512], st_ps[:],
                        Act.Exp, scale=SCALE,
                    )

                diag = list(range(4 * qc, nkt))
                plain = list(range(0, 4 * qc))
                korder = []
                step = max(1, (nkt + 3) // 4)
                pi = 0
                for d in diag:
                    korder.append(d)
                    take = plain[pi : pi + step - 1]
                    korder.extend(take)
                    pi += len(take)
                korder.extend(plain[pi:])

                def emit_pv(kt):
                    first = kt == korder[0]
                    last = kt == korder[-1]
                    nc.tensor.matmul(
                        sum_ps[:], ones_sb[:], pt[:, kt * 512 : (kt + 1) * 512],
                        start=first, stop=last,
                    )
                    nc.tensor.matmul(
                        attn_ps[:], v_nat[:, kt * 128 : (kt + 1) * 128],
                        pt[:, kt * 512 : (kt + 1) * 512],
                        start=first, stop=last,
                    )

                # diagonal (masked) tiles first so their longer mask+exp chain
                # is covered by later S work; PV trails S by 2.
                lag = min(3, nkt - 1)
                for i, kt in enumerate(korder):
                    emit_s(kt)
                    if i >= lag:
                        emit_pv(korder[i - lag])
                for i in range(nkt - lag, nkt):
                    emit_pv(korder[i])
                invb = sp.tile([128, 512], f32, tag="invb", bufs=2)
                # reciprocal on the idle Scalar engine (ACT LUT, ~2^-10 rel —
                # ample for a bf16 result); bass's activation() refuses
                # Reciprocal so emit a Copy and patch the func.
                _i = nc.scalar.activation(
                    invb[:], sum_ps[:], Act.Copy
                )
                _i.ins.func = Act.Reciprocal
                nc.vector.tensor_tensor(
                    out=attnT[qc][:, h * 512 : (h + 1) * 512], in0=attn_ps[:],
                    in1=invb[:], op=Alu.mult,
                )

            ro_locs = {}

            def phase_b_ro(qc):
                ro_ps = ps.tile([RANK, 512], f32, tag="q0", name="ro_ps")
                for j in range(QH):
                    nc.tensor.matmul(
                        ro_ps[:], ao_sb[:, j * RANK : (j + 1) * RANK],
                        attnT[qc][:, j * 512 : (j + 1) * 512],
                        start=(j == 0), stop=(j == QH - 1),
                    )
                ro_loc = sp.tile([RANK, 512], bf16, tag="roloc", bufs=2)
                nc.vector.tensor_copy(ro_loc[:], ro_ps[:])
                ro_locs[qc] = ro_loc

            def phase_ag(qc):
                nc.scalar.dma_start(
                    agin[qc][0:QO, :].rearrange("(j p) m -> p j m", p=128),
                    attnT[qc].rearrange("p (j m) -> p j m", j=QH),
                )
                nc.scalar.dma_start(agin[qc][QO:AGROWS, :], ro_locs.pop(qc)[:])
                nc.gpsimd.collective_compute(
                    "AllGather",
                    mybir.AluOpType.bypass,
                    replica_groups=[list(range(NC_))],
                    ins=[agin[qc].ap().opt()],
                    outs=[agout[qc].ap().opt()],
                )

            def phase_c(qc):
                q0 = qc * 512
                # alternate PSUM tag sets so consecutive chunks double-buffer
                otags = ("pk", "pv", "pr", "c8") if qc % 2 == 0 else ("q0", "q1", "q2", "q3")
                rsum_t = sp.tile([128, 512], bf16, tag="rsum", bufs=2)
                for rr in range(NC_):
                    nc.scalar.dma_start(
                        rsum_t[rr * RANK : (rr + 1) * RANK, :],
                        agout[qc][rr * AGROWS + QO : (rr + 1) * AGROWS, :],
                    )
                ro_ps2 = ps.tile([RANK, 512], f32, tag=otags[0], name="ro_ps2")
                nc.tensor.matmul(ro_ps2[:], ss_sb[:], rsum_t[:], start=True, stop=True)
                ro2 = sp.tile([RANK, 512], bf16, tag="ro2sb", bufs=2)
                nc.vector.tensor_copy(ro2[:], ro_ps2[:])
                out_ps = [
                    ps.tile([128, 512], f32, tag=t, name=f"out_ps_{t}")
                    for t in otags
                ]
                for r in range(NC_):
                    attg = sp.tile([128, 4 * 512], bf16, tag="att", bufs=4)
                    nc.sync.dma_start(
                        attg.rearrange("p (t m) -> p t m", t=4),
                        agout[qc][r * AGROWS : r * AGROWS + QO, :].rearrange(
                            "(t p) m -> p t m", p=128
                        ),
                    )
                    for j2 in range(4):
                        ot = r * 4 + j2
                        att = attg[:, j2 * 512 : (j2 + 1) * 512]
                        for j in range(4):
                            nc.tensor.matmul(
                                out_ps[j][:],
                                wo_sb[ot][:, j * 128 : (j + 1) * 128],
                                att,
                                start=(ot == 0),
                                stop=False,
                            )
                for j in range(4):
                    nc.tensor.matmul(
                        out_ps[j][:], bo_sb[:, j * 128 : (j + 1) * 128], ro2[:],
                        start=False, stop=True,
                    )
                ot_sb = sp.tile([128, 4 * 512], f32, tag="osb", bufs=1)
                for j in range(4):
                    nc.vector.tensor_copy(
                        ot_sb[:, j * 512 : (j + 1) * 512], out_ps[j][:]
                    )
                nc.scalar.dma_start(
                    out_d[:, q0 : q0 + 512].rearrange("(j p) m -> p j m", p=128),
                    ot_sb.rearrange("p (j m) -> p j m", j=4),
                )

            # schedule: fine-grained A/B interleave; each head's attention is
            # emitted right after its q tile's rope so the PE never waits for
            # the whole rope burst. AGs fire per chunk; C runs contiguously.
            for sc in range(NSC):
                s0 = sc * 512
                q_ps, k_ps = phase_a_mm(sc)
                if sc == NSC - 1:
                    wo_g = load_grouped(wo_t, QO, "bigw")
                    wo_sb.extend(
                        wo_g[dt // 4][:, (dt % 4) * QO : (dt % 4 + 1) * QO]
                        for dt in range(NDT)
                    )
                rope(k_ps, kT, s0)
                pt = ap_.tile([128, NKT * 512], bf16, tag="pt", name="pt", bufs=1)
                for h in range(QH):
                    rope(q_ps[h], qT[h], s0)
                    phase_b_head(sc, h, pt)
                    if h == 0 and sc >= 1:
                        # previous chunk's AllGather fires here so its HBM
                        # burst lands in this PE-bound region, not on the
                        # next chunk's xt stream.
                        phase_ag(sc - 1)
                phase_b_ro(sc)
            phase_ag(NSC - 1)
            for qc in range(NSC):
                phase_c(qc)

    _split_excess_waits(nc)
    return nc


def _host_prep(inputs):
    x = np.asarray(inputs["x"], np.float32)[0]  # [SEQ, DIM]
    wq = np.asarray(inputs["wq"], np.float32)
    wk = np.asarray(inputs["wk"], np.float32)
    wv = np.asarray(inputs["wv"], np.float32)
    wo = np.asarray(inputs["wo"], np.float32)
    fc = np.asarray(inputs["freqs_cos"], np.float32)  # [SEQ, 64]
    fs = np.asarray(inputs["freqs_sin"], np.float32)
    aq = np.asarray(inputs["lora_q_A"], np.float32)
    bq = np.asarray(inputs["lora_q_B"], np.float32)
    ak = np.asarray(inputs["lora_k_A"], np.float32)
    bk = np.asarray(inputs["lora_k_B"], np.float32)
    av = np.asarray(inputs["lora_v_A"], np.float32)
    bv = np.asarray(inputs["lora_v_B"], np.float32)
    ao = np.asarray(inputs["lora_o_A"], np.float32)
    bo = np.asarray(inputs["lora_o_B"], np.float32)

    # even/odd permutation inside each head block (RoPE layout)
    def perm(n_heads):
        p = []
        for h in range(n_heads):
            p.extend(h * HD + np.r_[0:HD:2])
            p.extend(h * HD + np.r_[1:HD:2])
        return np.array(p)

    pq, pk = perm(N_HEADS), perm(N_KV)
    wq_p, wk_p = wq[pq], wk[pk]
    bq_p, bk_p = bq[pq], bk[pk]

    xt = np.ascontiguousarray(x.T).astype(BF16)  # [DIM, SEQ]
    cos_t = np.ascontiguousarray(fc.T)  # [64, SEQ]
    sin_t = np.ascontiguousarray(fs.T)
    cos_d = np.concatenate([cos_t, cos_t], 0).astype(np.float32)
    sin_d = np.concatenate([sin_t, sin_t], 0).astype(np.float32)

    # mask tiles for the S^T diagonal chunks: [128 k, 4*512] f32
    kp = np.arange(128)[:, None]
    qp = np.arange(128)[None, :]
    tri = np.where(kp > qp, np.float32(-1e9), np.float32(0.0))  # [128k,128q]
    maskt = np.zeros((128, 4 * 512), np.float32)
    for jrel in range(4):
        blk = np.zeros((128, 512), np.float32)
        for jj in range(4):
            if jj < jrel:
                blk[:, jj * 128 : (jj + 1) * 128] = -1e9
            elif jj == jrel:
                blk[:, jj * 128 : (jj + 1) * 128] = tri
        maskt[:, jrel * 512 : (jrel + 1) * 512] = blk

    ident = np.eye(128, dtype=BF16)
    ones_c = np.ones((128, 128), BF16)
    sum_sel = np.zeros((128, RANK), np.float32)
    for r in range(NC_):
        for p in range(RANK):
            sum_sel[r * RANK + p, p] = 1.0
    sum_sel = sum_sel.astype(BF16)

    aqkv = np.zeros((96, DIM), np.float32)
    aqkv[0:RANK] = aq
    aqkv[32 : 32 + RANK] = ak
    aqkv[64 : 64 + RANK] = av
    aqkv_t = np.ascontiguousarray(aqkv.T).astype(BF16)

    shared = dict(
        xt=xt,
        aqkv_t=aqkv_t,
        cos_d=cos_d,
        sin_d=sin_d,
        maskt=maskt.astype(BF16),
        ident=ident,
        ones_c=ones_c,
        sum_sel=sum_sel,
    )
    in_maps = []
    for i in range(NC_):
        qs = slice(QO * i, QO * (i + 1))
        ks = slice(HD * i, HD * (i + 1))
        m = dict(shared)
        m["wq_t"] = np.ascontiguousarray(wq_p[qs].T).astype(BF16)
        m["wk_t"] = np.ascontiguousarray(wk_p[ks].T).astype(BF16)
        m["wv_t"] = np.ascontiguousarray(wv[ks].T).astype(BF16)
        m["bq_t"] = np.ascontiguousarray(bq_p[qs].T).astype(BF16)
        m["bk_t"] = np.ascontiguousarray(bk_p[ks].T).astype(BF16)
        m["bv_t"] = np.ascontiguousarray(bv[ks].T).astype(BF16)
        m["wo_t"] = np.ascontiguousarray(wo[qs].T).astype(BF16)
        # A_o^T rows for this core's local heads, laid out per o-tile
        aot = np.ascontiguousarray(ao.T[qs])  # [512, 16]
        m["ao_loc"] = np.ascontiguousarray(
            aot.reshape(QH, 128, RANK).transpose(1, 0, 2).reshape(128, QH * RANK)
        ).astype(BF16)
        m["bo_t"] = np.ascontiguousarray(bo[qs].T).astype(BF16)
        in_maps.append(m)
    return in_maps


def kernel(**inputs):
    _install_hooks()
    from concourse.bass_utils import run_bass_kernel_spmd

    if "nc" not in _CACHE:
        _CACHE["nc"] = _build()
    nc = _CACHE["nc"]

    in_maps = _host_prep(inputs)
    try:
        res = run_bass_kernel_spmd(
            nc, in_maps, core_ids=list(range(NC_)), trace=False
        )
    except Exception:
        # transient device-unrecoverable errors have been observed on the
        # first execution after a fresh compile; one retry clears them.
        res = run_bass_kernel_spmd(
            nc, in_maps, core_ids=list(range(NC_)), trace=False
        )
    out = np.empty((1, SEQ, DIM), np.float32)
    for i in range(NC_):
        out[0, :, QO * i : QO * (i + 1)] = res.results[i]["out"].T
    return out
